# revision 45
# baseline (speedup 1.0000x reference)
"""Causal temporal attention (B=4, T=2048, D=1024, H=16, hd=64) on 8 trn2 cores.

Sharding: core c handles batch b=c//2 and head-group hg=c%2 (8 heads, 512 dims).
Each core computes y_partial[b] = attn_out_g @ Wo_g.T for its head group; the
host sums the two partials per batch and adds bo.

Per-core dataflow:
  xT [1024, 2048] (host-pretransposed x[b]) streams in 256-col sub-chunks.
  qT,kT are computed transposed [512, T] (dims on partitions) so the S matmul
  contracts head_dim on partitions; v is computed natural [T, 512] with an
  appended ones-column per head so the AV matmul also produces the softmax
  denominator (row 64 of the [65, 512] accumulator).
  RMS-norm over head_dim (= partitions) uses a block-ones matmul for the
  sum-of-squares, ln/exp on ACT for rsqrt, and a broadcast matmul (with the
  norm weight folded in) to spread it back over partitions.
  Causality: tiles above the diagonal are skipped; boundary 128x128 blocks
  are masked by a triangular 0/1 multiply on GPSIMD after the exp.
All matmul inputs are float32r (TF32-like rounding, fp32 accumulation).
"""

import ml_dtypes
import numpy as np

import concourse.bass as bass
import concourse.tile as tile
from concourse import bacc, mybir
from concourse.bass_utils import run_bass_kernel_spmd
from concourse import bass2jax

F32 = mybir.dt.float32
F32R = mybir.dt.float32r
BF16 = mybir.dt.bfloat16
EXP = mybir.ActivationFunctionType.Exp
LN = mybir.ActivationFunctionType.Ln
COPY = mybir.ActivationFunctionType.Copy

EPS = 1e-6

# Force Ln and Exp onto the one ACT table set that contains both
# ("natural_log_exp_and_others"): the default first-match assignment puts
# them in different sets, and every Ln<->Exp transition then costs a ~2.7us
# table reload. Filtering (not reordering) keeps act_func_set_id positions
# valid for walrus.
_orig_gat = bacc.get_activation_tables


def _gat_combined(arch):
    tabs = _orig_gat(arch)
    drop = {mybir.ActivationFunctionType.Exp, mybir.ActivationFunctionType.Ln}
    return {
        name: (fns if name == "natural_log_exp_and_others" else fns - drop)
        for name, fns in tabs.items()
    }


bacc.get_activation_tables = _gat_combined


def build_module(T=2048, with_qkbias=False, with_vbias=False, n_cores=8):
    """Build the per-core Bass module. D=1024, 8 heads of 64 dims per core."""
    D = 1024
    HG = 8          # heads per core
    HD = 64         # head dim
    DG = HG * HD    # 512 group dims
    NKT = T // 128  # k/t tiles
    NCH = T // 512  # q chunks
    SUB = 256       # xT streaming sub-chunk width

    nc = bacc.Bacc("TRN2", target_bir_lowering=False, debug=False,
                   num_devices=n_cores)

    xT_d = nc.dram_tensor("xt", [D, T], BF16, kind="ExternalInput")
    wq_d = nc.dram_tensor("wq", [D, DG], BF16, kind="ExternalInput")
    wk_d = nc.dram_tensor("wk", [D, DG], BF16, kind="ExternalInput")
    wv_d = nc.dram_tensor("wv", [D, DG], BF16, kind="ExternalInput")
    wo_d = nc.dram_tensor("wo", [DG, D], BF16, kind="ExternalInput")
    tri_d = nc.dram_tensor("tri", [128, 128], BF16, kind="ExternalInput")
    blk_d = nc.dram_tensor("blk", [128, 3, 66], F32R, kind="ExternalInput")
    bcqk_d = nc.dram_tensor("bcqk", [66, 256], F32R, kind="ExternalInput")
    vones_d = nc.dram_tensor("vones", [128, HG], BF16, kind="ExternalInput")
    if with_qkbias:
        bq_d = nc.dram_tensor("bq", [4, 128], F32, kind="ExternalInput")
        bk_d = nc.dram_tensor("bk", [4, 128], F32, kind="ExternalInput")
    if with_vbias:
        bv_d = nc.dram_tensor("bv", [1, DG], F32R, kind="ExternalInput")
        ones1_d = nc.dram_tensor("ones1", [1, 128], F32R, kind="ExternalInput")
    y_d = nc.dram_tensor("y", [T, D], F32, kind="ExternalOutput")

    with nc.allow_low_precision(reason="float32r matmul inputs"), \
         tile.TileContext(nc) as tc:
        with (
            tc.tile_pool(name="res", bufs=1) as res,
            tc.tile_pool(name="ktp", bufs=1) as ktp,
            tc.tile_pool(name="vtp", bufs=1) as vtp,
            tc.tile_pool(name="st2", bufs=2) as st2,
            tc.tile_pool(name="st3", bufs=3) as st3,
            tc.tile_pool(name="st5", bufs=5) as st5,
            tc.tile_pool(name="qtp", bufs=2) as qtp,
            tc.tile_pool(name="psbig", bufs=2, space="PSUM") as psbig,
            tc.tile_pool(name="psmid", bufs=2, space="PSUM") as psmid,
            tc.tile_pool(name="pso", bufs=2, space="PSUM") as pso,
        ):
            # ---- resident loads ----
            # (xT chunk-0 and wq are hoisted first so the first projection
            # matmuls aren't stuck behind the full weight download)
            xT_ap = xT_d.ap().rearrange("(a p) t -> p a t", p=128)
            wq_sb = res.tile([128, 8, DG], BF16, tag="wq")
            wk_sb = res.tile([128, 8, DG], BF16, tag="wk")
            wv_sb = res.tile([128, 8, DG], BF16, tag="wv")
            wo_sb = res.tile([128, 4, D], BF16, tag="wo")
            wq_ap = wq_d.ap().rearrange("(a p) m -> p a m", p=128)
            # fine-grained startup: k-half pieces so the first projection
            # matmuls are gated on ~0.4MB of DMA instead of 2MB.
            xts0 = []
            for s in range(2):
                xt = st3.tile([128, 8, SUB], BF16, tag="xt", bufs=6,
                              name=f"xt0_{s}")
                xts0.append(xt)
            for s in range(2):
                for kh in range(2):
                    nc.sync.dma_start(
                        out=wq_sb[:, 4 * kh:4 * kh + 4, s * 128:(s + 1) * 128],
                        in_=wq_ap[:, 4 * kh:4 * kh + 4, s * 128:(s + 1) * 128])
                    nc.sync.dma_start(
                        out=xts0[s][:, 4 * kh:4 * kh + 4, :],
                        in_=xT_ap[:, 4 * kh:4 * kh + 4, s * SUB:(s + 1) * SUB])
            nc.sync.dma_start(out=wq_sb[:, :, 256:512], in_=wq_ap[:, :, 256:512])
            wk_ap = wk_d.ap().rearrange("(a p) m -> p a m", p=128)
            nc.sync.dma_start(out=wk_sb[:, :, 0:256], in_=wk_ap[:, :, 0:256])
            nc.sync.dma_start(out=wk_sb[:, :, 256:512], in_=wk_ap[:, :, 256:512])
            nc.sync.dma_start(out=wv_sb[:], in_=wv_d.ap().rearrange("(a p) m -> p a m", p=128))
            tri_sb = res.tile([128, 128], BF16, tag="tri")
            nc.sync.dma_start(out=tri_sb[:], in_=tri_d.ap())
            blk_sb = res.tile([128, 3, 66], F32R, tag="blk")
            nc.sync.dma_start(out=blk_sb[:], in_=blk_d.ap())
            bcqk_sb = res.tile([66, 256], F32R, tag="bcqk")
            nc.sync.dma_start(out=bcqk_sb[:], in_=bcqk_d.ap())
            vones_sb = res.tile([128, HG], BF16, tag="vones")
            nc.sync.dma_start(out=vones_sb[:], in_=vones_d.ap())
            eps_sb = res.tile([66, 1], F32, tag="eps")
            nc.vector.memset(eps_sb[:], EPS)
            ones64 = res.tile([1, 64], F32R, tag="ones64")
            nc.vector.memset(ones64[:], 1.0)
            nc.sync.dma_start(out=wo_sb[:], in_=wo_d.ap().rearrange("(a p) m -> p a m", p=128))
            bq_sb = bk_sb = bv_sb = ones1_sb = None
            if with_qkbias:
                bq_sb = res.tile([128, 4], F32, tag="bq")
                nc.sync.dma_start(out=bq_sb[:], in_=bq_d.ap().rearrange("m p -> p m"))
                bk_sb = res.tile([128, 4], F32, tag="bk")
                nc.sync.dma_start(out=bk_sb[:], in_=bk_d.ap().rearrange("m p -> p m"))
            if with_vbias:
                bv_sb = res.tile([1, DG], F32R, tag="bv")
                nc.sync.dma_start(out=bv_sb[:], in_=bv_d.ap())
                ones1_sb = res.tile([1, 128], F32R, tag="ones1")
                nc.sync.dma_start(out=ones1_sb[:], in_=ones1_d.ap())

            # resident kT [dims, T] (4 tiles) and v [t, dims+ones] (NKT tiles)
            kt_sb = [ktp.tile([128, T], BF16, tag=f"kt{m}", name=f"kt{m}") for m in range(4)]
            v_sb = [vtp.tile([128, HG, HD + 1], BF16, tag=f"v{t}", name=f"v{t}")
                    for t in range(NKT)]

            # filler queues: ~0.4-0.9us closures of pure PE work, popped
            # wherever the PE stream would otherwise stall (proj-phase DMA
            # waits, attention exp waits). vfills entries are (chunk, fn);
            # chunk cc's vfills must all be emitted by the end of attention
            # cc-1 (the diagonal AV of chunk cc reads its own v tiles).
            vfills = []
            outq = []
            xts_map = {0: xts0}

            def pop_fill(keep=0):
                if vfills:
                    vfills.pop(0)[1]()
                elif len(outq) > keep:
                    outq.pop(0)()

            def make_vfills(cc, xts_use):
                vstate = {}

                def half(tt, lo):
                    def emit():
                        s, toff = divmod(tt * 128, SUB)
                        if lo == 0:
                            vstate[tt] = psmid.tile(
                                [128, 512], F32, tag="mid",
                                name=f"vps{cc}_{tt}")
                        ps = vstate[tt]
                        for k in range(lo, lo + 4):
                            nc.tensor.matmul(
                                ps[:],
                                xts_use[s][:, k, toff:toff + 128],
                                wv_sb[:, k, :],
                                start=(k == 0),
                                stop=(k == 7 and not with_vbias),
                            )
                        if lo == 4:
                            if with_vbias:
                                nc.tensor.matmul(
                                    ps[:], ones1_sb[:], bv_sb[:],
                                    start=False, stop=True)
                            vt = v_sb[cc * 4 + tt]
                            nc.vector.tensor_copy(
                                vt[:, :, 0:HD],
                                ps[:].rearrange("p (h d) -> p h d", h=HG),
                            )
                            nc.sync.dma_start(
                                out=vt[:, :, HD:HD + 1],
                                in_=vones_sb[:].rearrange(
                                    "p (h o) -> p h o", o=1),
                            )
                    return emit

                return [half(tt, lo) for tt in range(4) for lo in (0, 4)]

            def stage(cc):
                """Prefetch chunk cc's xT and queue its v-proj fillers."""
                if cc >= NCH or cc in xts_map:
                    return
                tiles = []
                for s in range(2):
                    xt = st3.tile([128, 8, SUB], BF16, tag="xt", bufs=6,
                                  name=f"xtn{cc}_{s}")
                    c0 = cc * 512 + s * SUB
                    nc.sync.dma_start(out=xt[:],
                                      in_=xT_ap[:, :, c0:c0 + SUB])
                    tiles.append(xt)
                xts_map[cc] = tiles
                vfills.extend((cc, fn) for fn in make_vfills(cc, tiles))

            for c in range(NCH):
                # ---------- projection phase for chunk c ----------
                xts = xts_map[c]

                qt_c = [qtp.tile([128, 512], BF16, tag=f"qt{m}", name=f"qt{m}", bufs=1)
                        for m in range(4)]

                # projection pipeline stages, skewed so PE never waits on
                # the DVE/ACT legs of the rms-norm chain.
                praw, psq = {}, {}

                def proj_qk(u):
                    which, m = u
                    w_sb = wq_sb if which == "q" else wk_sb
                    b_sb = bq_sb if which == "q" else bk_sb
                    ps = psmid.tile([128, 512], F32, tag="mid")
                    for s in range(2):
                        for k in range(8):
                            nc.tensor.matmul(
                                ps[:, s * SUB:(s + 1) * SUB],
                                w_sb[:, k, m * 128:(m + 1) * 128],
                                xts[s][:, k, :],
                                start=(k == 0), stop=(k == 7),
                            )
                    raw = st5.tile([128, 512], F32, tag="praw", bufs=8)
                    if b_sb is not None:
                        nc.vector.tensor_scalar_add(raw[:], ps[:],
                                                    b_sb[:, m:m + 1])
                    else:
                        # psum->sbuf copies ride on ACT: DVE is the backlog
                        # engine at phase boundaries (recip chains + v/qt
                        # muls), and ACT has slack outside the exp bursts.
                        nc.scalar.activation(out=raw[:], in_=ps[:], func=COPY)
                    sq = st3.tile([128, 512], F32R, tag="sq", bufs=1)
                    nc.vector.tensor_mul(sq[:], raw[:], raw[:])
                    praw[u] = raw
                    psq[u] = sq

                # rsqrt staging: units packed 3-per-tile at 32-aligned
                # partition bases (matmul bases must be 0/32/64). The ln/exp
                # run over the whole [66, 512] tile; rows between the packed
                # pairs are junk that is never read.
                rs_tiles = [st2.tile([66, 512], F32R, tag=f"rs{j}",
                                     name=f"rs{j}", bufs=1) for j in range(3)]
                ssq3 = [None, None, None]

                def rs_slice(i):
                    return rs_tiles[i // 3][32 * (i % 3):32 * (i % 3) + 2, :]

                def sumsq(i, u):
                    g, j = divmod(i, 3)
                    if j == 0:
                        ssq3[g] = pso.tile([66, 512], F32, tag="o",
                                           name=f"ssq3_{g}")
                    last = i in (2, 5, 7)
                    # blk3[:, j] spreads unit j's sums to rows 32j:32j+2 and
                    # zeros elsewhere, so the accumulated tile is fully
                    # written before the ln reads it.
                    nc.tensor.matmul(ssq3[g][:], blk_sb[:, j, :],
                                     psq[u][:], start=(j == 0), stop=last)
                    if last:
                        # ln then rsqrt-exp immediately: the rs chain for
                        # group g completes while later units still project,
                        # so the first bcast_mul never waits on ACT.
                        nc.scalar.activation(out=rs_tiles[g][:],
                                             in_=ssq3[g][:], func=LN,
                                             bias=eps_sb[:], scale=1.0 / HD)
                        nc.scalar.activation(out=rs_tiles[g][:],
                                             in_=rs_tiles[g][:],
                                             func=EXP, scale=-0.5)

                def bcast_mul(i, u):
                    which, m = u
                    rsb = psbig.tile([128, 512], F32, tag="big")
                    b0 = 32 * (i % 3)
                    co = 0 if which == "q" else 128
                    nc.tensor.matmul(rsb[:],
                                     bcqk_sb[b0:b0 + 2, co:co + 128],
                                     rs_slice(i),
                                     start=True, stop=True)
                    if which == "q":
                        nc.vector.tensor_mul(qt_c[m][:], praw[u][:], rsb[:])
                    else:
                        nc.vector.tensor_mul(
                            kt_sb[m][:, c * 512:(c + 1) * 512],
                            praw[u][:], rsb[:])

                def proj_v(tt, cc, xts_use):
                    s, toff = divmod(tt * 128, SUB)
                    ps = psmid.tile([128, 512], F32, tag="mid")
                    for k in range(8):
                        nc.tensor.matmul(
                            ps[:],
                            xts_use[s][:, k, toff:toff + 128],
                            wv_sb[:, k, :],
                            start=(k == 0), stop=(k == 7 and not with_vbias),
                        )
                    if with_vbias:
                        nc.tensor.matmul(ps[:], ones1_sb[:], bv_sb[:],
                                         start=False, stop=True)
                    vt = v_sb[cc * 4 + tt]
                    nc.vector.tensor_copy(
                        vt[:, :, 0:HD],
                        ps[:].rearrange("p (h d) -> p h d", h=HG),
                    )
                    nc.sync.dma_start(
                        out=vt[:, :, HD:HD + 1],
                        in_=vones_sb[:].rearrange("p (h o) -> p h o", o=1),
                    )

                units = [("q", m) for m in range(4)] + [("k", m) for m in range(4)]
                # proj(u_i) skewed with sumsq(u_{i-1}); the previous chunk's
                # deferred out-projection interleaves here (queues are quiet);
                # then v tiles (PE work covering the ACT ln/exp latency);
                # then the 8 bcast+muls.
                # no pops in the unit loop: keep the mid psum ring free for
                # the projection pipeline. Fillers drain in the attention
                # phase, where the exp stream leaves PE slack.
                for i, u in enumerate(units):
                    proj_qk(u)
                    if i >= 1:
                        sumsq(i - 1, units[i - 1])
                sumsq(len(units) - 1, units[-1])
                # all 8 norm-broadcasts here in the proj phase: the big ring
                # is idle (attention hasn't started), so rsb never steals an
                # S-pipeline slot mid-attention.
                for mt in range(4):
                    bcast_mul(mt, ("q", mt))
                    bcast_mul(4 + mt, ("k", mt))
                if c == 0:
                    for tt in range(4):
                        proj_v(tt, 0, xts)

                # ---------- attention phase for chunk c ----------
                # prefetch TWO chunks ahead: chunk c+2's v-proj fillers give
                # the late stretch of this attention phase (after c+1's
                # fillers run out) more PE supply.
                stage(c + 1)
                stage(c + 2)
                # bufs=2: deferred out-projection closures may emit after the
                # NEXT chunk's attention starts writing its ot tiles; a ring
                # of 2 keeps the reads on the old slot.
                ot_c = [qtp.tile([128, 512], BF16, tag=f"ot{m}", name=f"ot{m}", bufs=2)
                        for m in range(4)]
                fill_tick = 0
                # last chunk: keep 2 out-proj fillers in the queue for the
                # final ot3 wait (they emit first in the tail drain).
                keep_c = 2 if c == NCH - 1 else 0

                def tick():
                    nonlocal fill_tick
                    fill_tick += 1
                    if fill_tick % 4 == 0:
                        pop_fill(keep_c)

                # Flattened attention pipeline: all heads' S/exp/AV groups in
                # one stream, with the AV lag carried ACROSS head boundaries
                # so the first AV of a head never waits on its own first exp.
                # Groups: per head, pairs of full k-tiles, then 2 diagonal
                # groups packing the 4 boundary tiles (masked post-exp).
                n_full = 4 * c
                chunk_groups = []
                for h in range(HG):
                    mt, r0 = h // 2, (h % 2) * 64
                    glist = []
                    for p0 in range(0, n_full, 2):
                        sm = [(0, p0, 0, 512), (512, p0 + 1, 0, 512)]
                        glist.append(dict(
                            smm=sm, etot=1024, mask=None,
                            av=[sm[0] + (p0 == 0, False),
                                sm[1] + (False, False)]))
                    ga = (0, n_full + 0, 0, 512)
                    gb = (512, n_full + 2, 256, 256)
                    glist.append(dict(
                        smm=[ga, gb], etot=768, mask=4,
                        av=[ga + (n_full == 0, False), gb + (False, False)]))
                    gc_ = (0, n_full + 1, 128, 384)
                    gd = (384, n_full + 3, 384, 128)
                    glist.append(dict(
                        smm=[gc_, gd], etot=512, mask=3,
                        av=[gc_ + (False, False), gd + (False, True)]))
                    for gi, g in enumerate(glist):
                        g.update(h=h, mt=mt, r0=r0,
                                 first_of_head=(gi == 0),
                                 last_of_head=(gi == len(glist) - 1))
                        chunk_groups.append(g)

                o_ps_map = {}

                def finish_head(g):
                    h, mt, r0 = g["h"], g["mt"], g["r0"]
                    o_ps = o_ps_map[h]
                    recip = st2.tile([1, 512], F32R, tag="recip", bufs=1)
                    nc.vector.reciprocal(out=recip[:], in_=o_ps[64:65, :])
                    if c == NCH - 1 and h == HG - 1:
                        # final head: the whole tail waits on this chain, so
                        # broadcast on PE (idle here) instead of GPSIMD.
                        rb_ps = pso.tile([64, 512], F32, tag="o",
                                         name="rb_ps")
                        nc.tensor.matmul(rb_ps[:], ones64[:], recip[:],
                                         start=True, stop=True)
                        nc.vector.tensor_mul(ot_c[mt][r0:r0 + 64, :],
                                             o_ps[0:64, :], rb_ps[:])
                    else:
                        recipb = st2.tile([64, 512], F32R, tag="recipb",
                                          bufs=1)
                        nc.gpsimd.partition_broadcast(recipb[:], recip[:])
                        nc.vector.tensor_mul(ot_c[mt][r0:r0 + 64, :],
                                             o_ps[0:64, :], recipb[:])

                def flush(p):
                    g, es = p
                    o_ps = o_ps_map[g["h"]]
                    for (col0, kt, q0, w, fi, la) in g["av"]:
                        nc.tensor.matmul(
                            o_ps[:, q0:q0 + w],
                            v_sb[kt][:, g["h"], :],
                            es[:, col0:col0 + w],
                            start=fi, stop=la,
                        )
                    if g["last_of_head"]:
                        finish_head(g)

                pend = None
                for g in chunk_groups:
                    if g["first_of_head"]:
                        pop_fill(keep_c)
                        o_ps_map[g["h"]] = pso.tile(
                            [HD + 1, 512], F32, tag="o",
                            name=f"o_ps{c}_{g['h']}")
                    sp = psbig.tile([128, 1024], F32, tag="big")
                    for (col0, kt, q0, width) in g["smm"]:
                        nc.tensor.matmul(
                            sp[:, col0:col0 + width],
                            kt_sb[g["mt"]][g["r0"]:g["r0"] + 64,
                                           kt * 128:(kt + 1) * 128],
                            qt_c[g["mt"]][g["r0"]:g["r0"] + 64,
                                          q0:q0 + width],
                            start=True, stop=True,
                        )
                    es = st3.tile([128, 1024], BF16, tag="es", bufs=3)
                    nc.scalar.activation(out=es[:, 0:g["etot"]],
                                         in_=sp[:, 0:g["etot"]],
                                         func=EXP, scale=0.125)
                    if g["mask"] is not None:
                        bstep = g["mask"]
                        esb = es[:].rearrange("p (a w) -> p a w", w=128)
                        nc.vector.tensor_mul(
                            esb[:, 0:bstep + 1:bstep, :],
                            esb[:, 0:bstep + 1:bstep, :],
                            tri_sb[:].rearrange("p (o w) -> p o w", o=1)
                            .to_broadcast((128, 2, 128)),
                        )
                    if pend is not None:
                        flush(pend)
                        tick()
                    pend = (g, es)
                flush(pend)
                # next chunk's attention needs its v tiles from the first
                # head's diagonal groups on: drain chunk c+1's leftovers
                # (chunk c+2's may linger into the next attention phase).
                while vfills and vfills[0][0] <= c + 1:
                    vfills.pop(0)[1]()

                # ---------- out-projection for chunk c (deferred) ----------
                def make_outproj(cc, ots):
                    def one(tt, od):
                        def emit():
                            # psmid while attention phases follow (psbig is
                            # the S-ring); for the LAST chunk alternate with
                            # psbig — its ring is past the final exps by
                            # allocation order, so 4 units pipeline the tail.
                            pool, tg = ((psbig, "big")
                                        if cc == NCH - 1 and (tt + od) % 2
                                        else (psmid, "mid"))
                            yp = pool.tile([128, 512], F32, tag=tg,
                                           name=f"yp{cc}_{tt}_{od}")
                            for m in range(4):
                                nc.tensor.matmul(
                                    yp[:],
                                    ots[m][:, tt * 128:(tt + 1) * 128],
                                    wo_sb[:, m, od * 512:(od + 1) * 512],
                                    start=(m == 0), stop=(m == 3),
                                )
                            ysb = st2.tile([128, 512], F32, tag="y", bufs=6,
                                           name=f"ysb{cc}_{tt}_{od}")
                            nc.scalar.activation(out=ysb[:], in_=yp[:],
                                                 func=COPY)
                            t0 = cc * 512 + tt * 128
                            nc.sync.dma_start(
                                out=y_d.ap()[t0:t0 + 128,
                                             od * 512:(od + 1) * 512],
                                in_=ysb[:])
                        return emit
                    return [one(tt, od) for tt in range(4) for od in range(2)]

                outq.extend(make_outproj(c, ot_c))
            while outq:
                outq.pop(0)()

    nc.compile()
    return nc


_CACHE = {}


def _get_module(T, with_qkbias, with_vbias, n_cores):
    key = (T, with_qkbias, with_vbias, n_cores)
    if key not in _CACHE:
        _CACHE[key] = build_module(T, with_qkbias, with_vbias, n_cores)
    return _CACHE[key]


def make_consts(qn_w, kn_w):
    HG = 8
    tri = np.triu(np.ones((128, 128), np.float32))   # keep k<=q: [i <= j]
    # blk[p, j, r] = 1 where r == 32j + p//64: unit-j sum-of-squares
    # selector covering all 66 output rows (zeros elsewhere).
    blk = np.zeros((128, 3, 66), np.float32)
    for j in range(3):
        blk[0:64, j, 32 * j] = 1.0
        blk[64:128, j, 32 * j + 1] = 1.0
    # broadcast lhsT replicated at partition bases 0/32/64 (PE needs
    # lhsT and rhs at the same base); cols 0:128 = qn, 128:256 = kn.
    bcqk = np.zeros((66, 256), np.float32)
    for j in range(3):
        for half in range(2):
            bcqk[32 * j + half, half * 64:(half + 1) * 64] = qn_w
            bcqk[32 * j + half, 128 + half * 64:128 + (half + 1) * 64] = kn_w
    vones = np.ones((128, HG), np.float32)
    return tri, blk, bcqk, vones


def make_in_maps(x, Wq, bq, Wk, bk, Wv, bv, Wo, qn_w, kn_w,
                 with_qkbias, with_vbias, n_cores=8):
    DG = 512
    tri, blk, bcqk, vones = make_consts(qn_w.astype(np.float32),
                                        kn_w.astype(np.float32))
    in_maps = []
    for c in range(n_cores):
        b, hg = divmod(c, 2)
        sl = slice(hg * DG, (hg + 1) * DG)
        bf = ml_dtypes.bfloat16
        im = {
            "xt": np.ascontiguousarray(x[b].T.astype(bf)),
            "wq": np.ascontiguousarray(Wq[sl, :].T.astype(bf)),
            "wk": np.ascontiguousarray(Wk[sl, :].T.astype(bf)),
            "wv": np.ascontiguousarray(Wv[sl, :].T.astype(bf)),
            "wo": np.ascontiguousarray(Wo[:, sl].T.astype(bf)),
            "tri": tri.astype(ml_dtypes.bfloat16), "blk": blk, "bcqk": bcqk,
            "vones": vones.astype(ml_dtypes.bfloat16),
        }
        if with_qkbias:
            im["bq"] = bq[sl].astype(np.float32).reshape(4, 128)
            im["bk"] = bk[sl].astype(np.float32).reshape(4, 128)
        if with_vbias:
            im["bv"] = bv[sl].astype(np.float32).reshape(1, DG)
            im["ones1"] = np.ones((1, 128), np.float32)
        in_maps.append(im)
    return in_maps


_RUNNER_CACHE = {}


def _run_cached(nc, in_maps, key):
    """run_bass_via_pjrt with the jitted executable cached across calls."""
    import jax
    from jax.sharding import Mesh, PartitionSpec
    from jax.experimental.shard_map import shard_map
    from concourse import mybir as _mb

    n_cores = len(in_maps)
    if key not in _RUNNER_CACHE:
        bass2jax.install_neuronx_cc_hook()
        part_name = (nc.partition_id_tensor.name
                     if nc.partition_id_tensor else None)
        in_names, out_names, out_avals = [], [], []
        for alloc in nc.m.functions[0].allocations:
            if not isinstance(alloc, _mb.MemoryLocationSet):
                continue
            name = alloc.memorylocations[0].name
            if alloc.kind == "ExternalInput":
                if name != part_name:
                    in_names.append(name)
            elif alloc.kind == "ExternalOutput":
                out_names.append(name)
                out_avals.append(jax.core.ShapedArray(
                    tuple(alloc.tensor_shape), _mb.dt.np(alloc.dtype)))
        n_params = len(in_names)
        all_names = in_names + out_names
        if part_name is not None:
            all_names = all_names + [part_name]

        def _body(*args):
            operands = list(args)
            if part_name is not None:
                operands.append(bass2jax.partition_id_tensor())
            outs = bass2jax._bass_exec_p.bind(
                *operands, out_avals=tuple(out_avals),
                in_names=tuple(all_names), out_names=tuple(out_names),
                lowering_input_output_aliases=(),
                sim_require_finite=True, sim_require_nnan=True, nc=nc)
            return tuple(outs)

        devices = jax.devices()[:n_cores]
        mesh = Mesh(np.asarray(devices), ("core",))
        n_outs = len(out_names)
        sharded = jax.jit(
            shard_map(_body, mesh=mesh,
                      in_specs=(PartitionSpec("core"),) * (n_params + n_outs),
                      out_specs=(PartitionSpec("core"),) * n_outs,
                      check_rep=False),
            donate_argnums=tuple(range(n_params, n_params + n_outs)),
            keep_unused=True)
        _RUNNER_CACHE[key] = (sharded, in_names, out_names, out_avals)

    sharded, in_names, out_names, out_avals = _RUNNER_CACHE[key]
    concat_in = [np.concatenate([np.asarray(m[nm]) for m in in_maps], axis=0)
                 for nm in in_names]
    concat_zeros = [np.zeros((n_cores * a.shape[0], *a.shape[1:]), a.dtype)
                    for a in out_avals]
    out_arrs = sharded(*concat_in, *concat_zeros)
    return [
        {nm: np.asarray(out_arrs[i]).reshape(n_cores, *out_avals[i].shape)[c]
         for i, nm in enumerate(out_names)}
        for c in range(n_cores)
    ]


def kernel(x, Wq, bq, Wk, bk, Wv, bv, Wo, bo, qn_w, kn_w):
    x = np.asarray(x); Wq = np.asarray(Wq); Wk = np.asarray(Wk)
    Wv = np.asarray(Wv); Wo = np.asarray(Wo)
    bq = np.asarray(bq); bk = np.asarray(bk); bv = np.asarray(bv)
    bo = np.asarray(bo)
    qn_w = np.asarray(qn_w); kn_w = np.asarray(kn_w)
    B, T, D = x.shape

    with_qkbias = bool(np.any(bq != 0) or np.any(bk != 0))
    with_vbias = bool(np.any(bv != 0))
    nc = _get_module(T, with_qkbias, with_vbias, 8)
    in_maps = make_in_maps(x, Wq, bq, Wk, bk, Wv, bv, Wo, qn_w, kn_w,
                           with_qkbias, with_vbias, 8)
    key = (T, with_qkbias, with_vbias, 8)
    results = _run_cached(nc, in_maps, key)
    out = np.empty((B, T, D), np.float32)
    for b in range(B):
        out[b] = results[2 * b]["y"] + results[2 * b + 1]["y"]
    out += bo.astype(np.float32)
    return out



# revision 56
# speedup vs baseline: 1.0357x; 1.0357x over previous
"""Causal temporal attention (B=4, T=2048, D=1024, H=16, hd=64) on 8 trn2 cores.

Sharding: core c handles batch b=c//2 and head-group hg=c%2 (8 heads, 512 dims).
Each core computes y_partial[b] = attn_out_g @ Wo_g.T for its head group; the
host sums the two partials per batch and adds bo.

Per-core dataflow:
  xT [1024, 2048] (host-pretransposed x[b]) streams in 256-col sub-chunks.
  qT,kT are computed transposed [512, T] (dims on partitions) so the S matmul
  contracts head_dim on partitions; v is computed natural [T, 512] with an
  appended ones-column per head so the AV matmul also produces the softmax
  denominator (row 64 of the [65, 512] accumulator).
  RMS-norm over head_dim (= partitions) uses a block-ones matmul for the
  sum-of-squares, ln/exp on ACT for rsqrt, and a broadcast matmul (with the
  norm weight folded in) to spread it back over partitions.
  Causality: tiles above the diagonal are skipped; boundary 128x128 blocks
  are masked by a triangular 0/1 multiply on GPSIMD after the exp.
All matmul inputs are float32r (TF32-like rounding, fp32 accumulation).
"""

import ml_dtypes
import numpy as np

import concourse.bass as bass
import concourse.tile as tile
from concourse import bacc, mybir
from concourse.bass_utils import run_bass_kernel_spmd
from concourse import bass2jax

F32 = mybir.dt.float32
F32R = mybir.dt.float32r
BF16 = mybir.dt.bfloat16
FP8 = mybir.dt.float8e4
DR = mybir.MatmulPerfMode.DoubleRow
EXP = mybir.ActivationFunctionType.Exp
LN = mybir.ActivationFunctionType.Ln
COPY = mybir.ActivationFunctionType.Copy

EPS = 1e-6

# Force Ln and Exp onto the one ACT table set that contains both
# ("natural_log_exp_and_others"): the default first-match assignment puts
# them in different sets, and every Ln<->Exp transition then costs a ~2.7us
# table reload. Filtering (not reordering) keeps act_func_set_id positions
# valid for walrus.
_orig_gat = bacc.get_activation_tables


def _gat_combined(arch):
    tabs = _orig_gat(arch)
    drop = {mybir.ActivationFunctionType.Exp, mybir.ActivationFunctionType.Ln}
    return {
        name: (fns if name == "natural_log_exp_and_others" else fns - drop)
        for name, fns in tabs.items()
    }


bacc.get_activation_tables = _gat_combined


def build_module(T=2048, with_qkbias=False, with_vbias=False, n_cores=8):
    """Build the per-core Bass module. D=1024, 8 heads of 64 dims per core."""
    D = 1024
    HG = 8          # heads per core
    HD = 64         # head dim
    DG = HG * HD    # 512 group dims
    NKT = T // 128  # k/t tiles
    NCH = T // 512  # q chunks
    SUB = 256       # xT streaming sub-chunk width

    nc = bacc.Bacc("TRN2", target_bir_lowering=False, debug=False,
                   num_devices=n_cores)

    # fp8 DoubleRow projections: x scaled x8, W scaled x32 (values centered
    # in e4m3 range), decomposed host-side into hi + residual-lo. Main
    # matmuls contract k-chunk PAIRS of hi*hi; correction matmuls pack the
    # (hi*lo + lo*hi) cross terms. psum = 256*(x@W); q/k renormalize via
    # rmsnorm, v's 256 cancels against a 256-valued denominator column.
    xm_d = nc.dram_tensor("xm", [4, 2, 128, T], FP8, kind="ExternalInput")
    xc_d = nc.dram_tensor("xc", [8, 2, 128, T], FP8, kind="ExternalInput")
    wqm_d = nc.dram_tensor("wqm", [4, 2, 128, DG], FP8, kind="ExternalInput")
    wqc_d = nc.dram_tensor("wqc", [8, 2, 128, DG], FP8, kind="ExternalInput")
    wkm_d = nc.dram_tensor("wkm", [4, 2, 128, DG], FP8, kind="ExternalInput")
    wkc_d = nc.dram_tensor("wkc", [8, 2, 128, DG], FP8, kind="ExternalInput")
    wvm_d = nc.dram_tensor("wvm", [4, 2, 128, DG], FP8, kind="ExternalInput")
    wvc_d = nc.dram_tensor("wvc", [8, 2, 128, DG], FP8, kind="ExternalInput")
    wo_d = nc.dram_tensor("wo", [DG, D], BF16, kind="ExternalInput")
    tri_d = nc.dram_tensor("tri", [128, 128], BF16, kind="ExternalInput")
    blk_d = nc.dram_tensor("blk", [128, 3, 66], F32R, kind="ExternalInput")
    bcqk_d = nc.dram_tensor("bcqk", [66, 256], F32R, kind="ExternalInput")
    vones_d = nc.dram_tensor("vones", [128, HG], BF16, kind="ExternalInput")
    if with_qkbias:
        bq_d = nc.dram_tensor("bq", [4, 128], F32, kind="ExternalInput")
        bk_d = nc.dram_tensor("bk", [4, 128], F32, kind="ExternalInput")
    if with_vbias:
        bv_d = nc.dram_tensor("bv", [1, DG], F32R, kind="ExternalInput")
        ones1_d = nc.dram_tensor("ones1", [1, 128], F32R, kind="ExternalInput")
    y_d = nc.dram_tensor("y", [T, D], F32, kind="ExternalOutput")

    with nc.allow_low_precision(reason="float32r matmul inputs"), \
         tile.TileContext(nc) as tc:
        with (
            tc.tile_pool(name="res", bufs=1) as res,
            tc.tile_pool(name="ktp", bufs=1) as ktp,
            tc.tile_pool(name="vtp", bufs=1) as vtp,
            tc.tile_pool(name="st2", bufs=2) as st2,
            tc.tile_pool(name="st3", bufs=3) as st3,
            tc.tile_pool(name="st5", bufs=5) as st5,
            tc.tile_pool(name="qtp", bufs=2) as qtp,
            tc.tile_pool(name="psbig", bufs=2, space="PSUM") as psbig,
            tc.tile_pool(name="psmid", bufs=2, space="PSUM") as psmid,
            tc.tile_pool(name="pso", bufs=2, space="PSUM") as pso,
        ):
            # ---- resident loads ----
            # (xT chunk-0 and wq are hoisted first so the first projection
            # matmuls aren't stuck behind the full weight download)
            xm_ap = xm_d.ap().rearrange("a b p t -> p a b t")
            xc_ap = xc_d.ap().rearrange("a b p t -> p a b t")
            wqm_sb = res.tile([128, 4, 2, DG], FP8, tag="wqm")
            wqc_sb = res.tile([128, 8, 2, DG], FP8, tag="wqc")
            wkm_sb = res.tile([128, 4, 2, DG], FP8, tag="wkm")
            wkc_sb = res.tile([128, 8, 2, DG], FP8, tag="wkc")
            wvm_sb = res.tile([128, 4, 2, DG], FP8, tag="wvm")
            wvc_sb = res.tile([128, 8, 2, DG], FP8, tag="wvc")
            wo_sb = res.tile([128, 4, D], BF16, tag="wo")
            wqm_ap = wqm_d.ap().rearrange("a b p m -> p a b m")
            wqc_ap = wqc_d.ap().rearrange("a b p m -> p a b m")
            # fine-grained startup: per-k-pair pieces so the first projection
            # matmuls are gated on ~0.3MB of DMA instead of 3MB.
            xm0 = st3.tile([128, 4, 2, 512], FP8, tag="xm", bufs=3,
                           name="xm0")
            xc0 = st3.tile([128, 8, 2, 512], FP8, tag="xc", bufs=3,
                           name="xc0")
            for kp in range(4):
                nc.sync.dma_start(out=wqm_sb[:, kp], in_=wqm_ap[:, kp])
                nc.sync.dma_start(out=xm0[:, kp], in_=xm_ap[:, kp, :, 0:512])
            for kh in range(2):
                nc.sync.dma_start(out=wqc_sb[:, 4 * kh:4 * kh + 4],
                                  in_=wqc_ap[:, 4 * kh:4 * kh + 4])
                nc.sync.dma_start(out=xc0[:, 4 * kh:4 * kh + 4],
                                  in_=xc_ap[:, 4 * kh:4 * kh + 4, :, 0:512])
            xts0 = (xm0, xc0)
            nc.sync.dma_start(out=wkm_sb[:], in_=wkm_d.ap().rearrange("a b p m -> p a b m"))
            nc.sync.dma_start(out=wkc_sb[:], in_=wkc_d.ap().rearrange("a b p m -> p a b m"))
            nc.sync.dma_start(out=wvm_sb[:], in_=wvm_d.ap().rearrange("a b p m -> p a b m"))
            nc.sync.dma_start(out=wvc_sb[:], in_=wvc_d.ap().rearrange("a b p m -> p a b m"))
            tri_sb = res.tile([128, 128], BF16, tag="tri")
            nc.sync.dma_start(out=tri_sb[:], in_=tri_d.ap())
            blk_sb = res.tile([128, 3, 66], F32R, tag="blk")
            nc.sync.dma_start(out=blk_sb[:], in_=blk_d.ap())
            bcqk_sb = res.tile([66, 256], F32R, tag="bcqk")
            nc.sync.dma_start(out=bcqk_sb[:], in_=bcqk_d.ap())
            vones_sb = res.tile([128, HG], BF16, tag="vones")
            nc.sync.dma_start(out=vones_sb[:], in_=vones_d.ap())
            eps_sb = res.tile([66, 1], F32, tag="eps")
            nc.vector.memset(eps_sb[:], EPS)
            ones64 = res.tile([1, 64], F32R, tag="ones64")
            nc.vector.memset(ones64[:], 1.0)
            nc.sync.dma_start(out=wo_sb[:], in_=wo_d.ap().rearrange("(a p) m -> p a m", p=128))
            bq_sb = bk_sb = bv_sb = ones1_sb = None
            if with_qkbias:
                bq_sb = res.tile([128, 4], F32, tag="bq")
                nc.sync.dma_start(out=bq_sb[:], in_=bq_d.ap().rearrange("m p -> p m"))
                bk_sb = res.tile([128, 4], F32, tag="bk")
                nc.sync.dma_start(out=bk_sb[:], in_=bk_d.ap().rearrange("m p -> p m"))
            if with_vbias:
                bv_sb = res.tile([1, DG], F32R, tag="bv")
                nc.sync.dma_start(out=bv_sb[:], in_=bv_d.ap())
                ones1_sb = res.tile([1, 128], F32R, tag="ones1")
                nc.sync.dma_start(out=ones1_sb[:], in_=ones1_d.ap())

            # resident kT [dims, T] (4 tiles) and v [t, dims+ones] (NKT tiles)
            kt_sb = [ktp.tile([128, T], BF16, tag=f"kt{m}", name=f"kt{m}") for m in range(4)]
            v_sb = [vtp.tile([128, HG, HD + 1], BF16, tag=f"v{t}", name=f"v{t}")
                    for t in range(NKT)]

            # filler queues: ~0.4-0.9us closures of pure PE work, popped
            # wherever the PE stream would otherwise stall (proj-phase DMA
            # waits, attention exp waits). vfills entries are (chunk, fn);
            # chunk cc's vfills must all be emitted by the end of attention
            # cc-1 (the diagonal AV of chunk cc reads its own v tiles).
            vfills = []
            outq = []
            xts_map = {0: xts0}

            def pop_fill(keep=0):
                if vfills:
                    vfills.pop(0)[1]()
                elif len(outq) > keep:
                    outq.pop(0)()

            def make_vfills(cc, xts_use):
                """v = x@Wv in fp8 DoubleRow, split in two ~0.9us halves:
                half 0 = main hi*hi over k-pairs, half 1 = corrections."""
                xm_t, xc_t = xts_use
                vstate = {}

                def half(tt, lo):
                    def emit():
                        toff = tt * 128
                        if lo == 0:
                            vstate[tt] = psmid.tile(
                                [128, 512], F32, tag="mid",
                                name=f"vps{cc}_{tt}")
                            ps = vstate[tt]
                            for kp in range(4):
                                nc.tensor.matmul(
                                    ps[:], xm_t[:, kp, :, toff:toff + 128],
                                    wvm_sb[:, kp], perf_mode=DR,
                                    start=(kp == 0), stop=False)
                        else:
                            ps = vstate[tt]
                            for k in range(8):
                                nc.tensor.matmul(
                                    ps[:], xc_t[:, k, :, toff:toff + 128],
                                    wvc_sb[:, k], perf_mode=DR,
                                    start=False,
                                    stop=(k == 7 and not with_vbias))
                            if with_vbias:
                                nc.tensor.matmul(
                                    ps[:], ones1_sb[:], bv_sb[:],
                                    start=False, stop=True)
                            vt = v_sb[cc * 4 + tt]
                            nc.vector.tensor_copy(
                                vt[:, :, 0:HD],
                                ps[:].rearrange("p (h d) -> p h d", h=HG),
                            )
                            nc.sync.dma_start(
                                out=vt[:, :, HD:HD + 1],
                                in_=vones_sb[:].rearrange(
                                    "p (h o) -> p h o", o=1),
                            )
                    return emit

                return [half(tt, lo) for tt in range(4) for lo in (0, 4)]

            def stage(cc):
                """Prefetch chunk cc's x (fp8 main+corr) and queue its
                v-proj fillers."""
                if cc >= NCH or cc in xts_map:
                    return
                c0 = cc * 512
                xm_t = st3.tile([128, 4, 2, 512], FP8, tag="xm", bufs=3,
                                name=f"xm{cc}")
                nc.sync.dma_start(out=xm_t[:], in_=xm_ap[:, :, :, c0:c0 + 512])
                xc_t = st3.tile([128, 8, 2, 512], FP8, tag="xc", bufs=3,
                                name=f"xc{cc}")
                nc.sync.dma_start(out=xc_t[:], in_=xc_ap[:, :, :, c0:c0 + 512])
                xts_map[cc] = (xm_t, xc_t)
                vfills.extend((cc, fn) for fn in make_vfills(cc, (xm_t, xc_t)))

            for c in range(NCH):
                # ---------- projection phase for chunk c ----------
                xts = xts_map[c]

                qt_c = [qtp.tile([128, 512], BF16, tag=f"qt{m}", name=f"qt{m}", bufs=1)
                        for m in range(4)]

                # projection pipeline stages, skewed so PE never waits on
                # the DVE/ACT legs of the rms-norm chain.
                praw, psq = {}, {}

                def proj_qk(u):
                    which, m = u
                    wm = wqm_sb if which == "q" else wkm_sb
                    wc = wqc_sb if which == "q" else wkc_sb
                    b_sb = bq_sb if which == "q" else bk_sb
                    xm_t, xc_t = xts
                    mc = slice(m * 128, (m + 1) * 128)
                    ps = psmid.tile([128, 512], F32, tag="mid")
                    for kp in range(4):
                        nc.tensor.matmul(ps[:], wm[:, kp, :, mc],
                                         xm_t[:, kp], perf_mode=DR,
                                         start=(kp == 0), stop=False)
                    for k in range(8):
                        nc.tensor.matmul(ps[:], wc[:, k, :, mc],
                                         xc_t[:, k], perf_mode=DR,
                                         start=False, stop=(k == 7))
                    raw = st5.tile([128, 512], F32, tag="praw", bufs=8)
                    if b_sb is not None:
                        nc.vector.tensor_scalar_add(raw[:], ps[:],
                                                    b_sb[:, m:m + 1])
                    else:
                        # psum->sbuf copies ride on ACT: DVE is the backlog
                        # engine at phase boundaries (recip chains + v/qt
                        # muls), and ACT has slack outside the exp bursts.
                        nc.scalar.activation(out=raw[:], in_=ps[:], func=COPY)
                    sq = st3.tile([128, 512], F32R, tag="sq", bufs=1)
                    nc.vector.tensor_mul(sq[:], raw[:], raw[:])
                    praw[u] = raw
                    psq[u] = sq

                # rsqrt staging: units packed 3-per-tile at 32-aligned
                # partition bases (matmul bases must be 0/32/64). The ln/exp
                # run over the whole [66, 512] tile; rows between the packed
                # pairs are junk that is never read.
                rs_tiles = [st2.tile([66, 512], F32R, tag=f"rs{j}",
                                     name=f"rs{j}", bufs=1) for j in range(3)]
                ssq3 = [None, None, None]

                def rs_slice(i):
                    return rs_tiles[i // 3][32 * (i % 3):32 * (i % 3) + 2, :]

                def sumsq(i, u):
                    g, j = divmod(i, 3)
                    if j == 0:
                        ssq3[g] = pso.tile([66, 512], F32, tag="o",
                                           name=f"ssq3_{g}")
                    last = i in (2, 5, 7)
                    # blk3[:, j] spreads unit j's sums to rows 32j:32j+2 and
                    # zeros elsewhere, so the accumulated tile is fully
                    # written before the ln reads it.
                    nc.tensor.matmul(ssq3[g][:], blk_sb[:, j, :],
                                     psq[u][:], start=(j == 0), stop=last)
                    if last:
                        # ln then rsqrt-exp immediately: the rs chain for
                        # group g completes while later units still project,
                        # so the first bcast_mul never waits on ACT.
                        nc.scalar.activation(out=rs_tiles[g][:],
                                             in_=ssq3[g][:], func=LN,
                                             bias=eps_sb[:], scale=1.0 / HD)
                        nc.scalar.activation(out=rs_tiles[g][:],
                                             in_=rs_tiles[g][:],
                                             func=EXP, scale=-0.5)

                def bcast_mul(i, u):
                    which, m = u
                    rsb = psbig.tile([128, 512], F32, tag="big")
                    b0 = 32 * (i % 3)
                    co = 0 if which == "q" else 128
                    nc.tensor.matmul(rsb[:],
                                     bcqk_sb[b0:b0 + 2, co:co + 128],
                                     rs_slice(i),
                                     start=True, stop=True)
                    if which == "q":
                        nc.vector.tensor_mul(qt_c[m][:], praw[u][:], rsb[:])
                    else:
                        nc.vector.tensor_mul(
                            kt_sb[m][:, c * 512:(c + 1) * 512],
                            praw[u][:], rsb[:])

                def proj_v(tt, cc, xts_use):
                    xm_t, xc_t = xts_use
                    toff = tt * 128
                    ps = psmid.tile([128, 512], F32, tag="mid")
                    for kp in range(4):
                        nc.tensor.matmul(
                            ps[:], xm_t[:, kp, :, toff:toff + 128],
                            wvm_sb[:, kp], perf_mode=DR,
                            start=(kp == 0), stop=False)
                    for k in range(8):
                        nc.tensor.matmul(
                            ps[:], xc_t[:, k, :, toff:toff + 128],
                            wvc_sb[:, k], perf_mode=DR,
                            start=False, stop=(k == 7 and not with_vbias))
                    if with_vbias:
                        nc.tensor.matmul(ps[:], ones1_sb[:], bv_sb[:],
                                         start=False, stop=True)
                    vt = v_sb[cc * 4 + tt]
                    nc.vector.tensor_copy(
                        vt[:, :, 0:HD],
                        ps[:].rearrange("p (h d) -> p h d", h=HG),
                    )
                    nc.sync.dma_start(
                        out=vt[:, :, HD:HD + 1],
                        in_=vones_sb[:].rearrange("p (h o) -> p h o", o=1),
                    )

                units = [("q", m) for m in range(4)] + [("k", m) for m in range(4)]
                # proj(u_i) skewed with sumsq(u_{i-1}); the previous chunk's
                # deferred out-projection interleaves here (queues are quiet);
                # then v tiles (PE work covering the ACT ln/exp latency);
                # then the 8 bcast+muls.
                # no pops in the unit loop: keep the mid psum ring free for
                # the projection pipeline. Fillers drain in the attention
                # phase, where the exp stream leaves PE slack.
                for i, u in enumerate(units):
                    proj_qk(u)
                    if i >= 1:
                        sumsq(i - 1, units[i - 1])
                sumsq(len(units) - 1, units[-1])
                # all 8 norm-broadcasts here in the proj phase: the big ring
                # is idle (attention hasn't started), so rsb never steals an
                # S-pipeline slot mid-attention.
                for mt in range(4):
                    bcast_mul(mt, ("q", mt))
                    bcast_mul(4 + mt, ("k", mt))
                if c == 0:
                    for tt in range(4):
                        proj_v(tt, 0, xts)

                # ---------- attention phase for chunk c ----------
                # prefetch TWO chunks ahead: chunk c+2's v-proj fillers give
                # the late stretch of this attention phase (after c+1's
                # fillers run out) more PE supply.
                stage(c + 1)
                stage(c + 2)
                # bufs=2: deferred out-projection closures may emit after the
                # NEXT chunk's attention starts writing its ot tiles; a ring
                # of 2 keeps the reads on the old slot.
                ot_c = [qtp.tile([128, 512], BF16, tag=f"ot{m}", name=f"ot{m}", bufs=2)
                        for m in range(4)]
                fill_tick = 0
                # last chunk: keep 2 out-proj fillers in the queue for the
                # final ot3 wait (they emit first in the tail drain).
                keep_c = 2 if c == NCH - 1 else 0

                def tick():
                    nonlocal fill_tick
                    fill_tick += 1
                    if fill_tick % 4 == 0:
                        pop_fill(keep_c)

                # Flattened attention pipeline: all heads' S/exp/AV groups in
                # one stream, with the AV lag carried ACROSS head boundaries
                # so the first AV of a head never waits on its own first exp.
                # Groups: per head, pairs of full k-tiles, then 2 diagonal
                # groups packing the 4 boundary tiles (masked post-exp).
                n_full = 4 * c
                chunk_groups = []
                for h in range(HG):
                    mt, r0 = h // 2, (h % 2) * 64
                    glist = []
                    for p0 in range(0, n_full, 2):
                        sm = [(0, p0, 0, 512), (512, p0 + 1, 0, 512)]
                        glist.append(dict(
                            smm=sm, etot=1024, mask=None,
                            av=[sm[0] + (p0 == 0, False),
                                sm[1] + (False, False)]))
                    ga = (0, n_full + 0, 0, 512)
                    gb = (512, n_full + 2, 256, 256)
                    glist.append(dict(
                        smm=[ga, gb], etot=768, mask=4,
                        av=[ga + (n_full == 0, False), gb + (False, False)]))
                    gc_ = (0, n_full + 1, 128, 384)
                    gd = (384, n_full + 3, 384, 128)
                    glist.append(dict(
                        smm=[gc_, gd], etot=512, mask=3,
                        av=[gc_ + (False, False), gd + (False, True)]))
                    for gi, g in enumerate(glist):
                        g.update(h=h, mt=mt, r0=r0,
                                 first_of_head=(gi == 0),
                                 last_of_head=(gi == len(glist) - 1))
                        chunk_groups.append(g)

                o_ps_map = {}

                def finish_head(g):
                    h, mt, r0 = g["h"], g["mt"], g["r0"]
                    o_ps = o_ps_map[h]
                    recip = st2.tile([1, 512], F32R, tag="recip", bufs=1)
                    nc.vector.reciprocal(out=recip[:], in_=o_ps[64:65, :])
                    if c == NCH - 1 and h == HG - 1:
                        # final head: the whole tail waits on this chain, so
                        # broadcast on PE (idle here) instead of GPSIMD. DVE
                        # may read only ONE psum operand, so stage o_ps rows
                        # to SBUF on ACT (concurrent with the recip).
                        rb_ps = pso.tile([64, 512], F32, tag="o",
                                         name="rb_ps")
                        nc.tensor.matmul(rb_ps[:], ones64[:], recip[:],
                                         start=True, stop=True)
                        osb = st2.tile([64, 512], F32R, tag="recipb",
                                       bufs=1, name="osb_last")
                        nc.scalar.activation(out=osb[:], in_=o_ps[0:64, :],
                                             func=COPY)
                        nc.vector.tensor_mul(ot_c[mt][r0:r0 + 64, :],
                                             osb[:], rb_ps[:])
                    else:
                        recipb = st2.tile([64, 512], F32R, tag="recipb",
                                          bufs=1)
                        nc.gpsimd.partition_broadcast(recipb[:], recip[:])
                        nc.vector.tensor_mul(ot_c[mt][r0:r0 + 64, :],
                                             o_ps[0:64, :], recipb[:])

                def flush(p):
                    g, es = p
                    o_ps = o_ps_map[g["h"]]
                    for (col0, kt, q0, w, fi, la) in g["av"]:
                        nc.tensor.matmul(
                            o_ps[:, q0:q0 + w],
                            v_sb[kt][:, g["h"], :],
                            es[:, col0:col0 + w],
                            start=fi, stop=la,
                        )
                    if g["last_of_head"]:
                        finish_head(g)

                pend = None
                for g in chunk_groups:
                    if g["first_of_head"]:
                        pop_fill(keep_c)
                        o_ps_map[g["h"]] = pso.tile(
                            [HD + 1, 512], F32, tag="o",
                            name=f"o_ps{c}_{g['h']}")
                    sp = psbig.tile([128, 1024], F32, tag="big")
                    for (col0, kt, q0, width) in g["smm"]:
                        nc.tensor.matmul(
                            sp[:, col0:col0 + width],
                            kt_sb[g["mt"]][g["r0"]:g["r0"] + 64,
                                           kt * 128:(kt + 1) * 128],
                            qt_c[g["mt"]][g["r0"]:g["r0"] + 64,
                                          q0:q0 + width],
                            start=True, stop=True,
                        )
                    es = st3.tile([128, 1024], BF16, tag="es", bufs=3)
                    nc.scalar.activation(out=es[:, 0:g["etot"]],
                                         in_=sp[:, 0:g["etot"]],
                                         func=EXP, scale=0.125)
                    if g["mask"] is not None:
                        bstep = g["mask"]
                        esb = es[:].rearrange("p (a w) -> p a w", w=128)
                        nc.vector.tensor_mul(
                            esb[:, 0:bstep + 1:bstep, :],
                            esb[:, 0:bstep + 1:bstep, :],
                            tri_sb[:].rearrange("p (o w) -> p o w", o=1)
                            .to_broadcast((128, 2, 128)),
                        )
                    if pend is not None:
                        flush(pend)
                        tick()
                    pend = (g, es)
                flush(pend)
                # next chunk's attention needs its v tiles from the first
                # head's diagonal groups on: drain chunk c+1's leftovers
                # (chunk c+2's may linger into the next attention phase).
                while vfills and vfills[0][0] <= c + 1:
                    vfills.pop(0)[1]()

                # ---------- out-projection for chunk c (deferred) ----------
                def make_outproj(cc, ots):
                    def one(tt, od):
                        def emit():
                            # psmid while attention phases follow (psbig is
                            # the S-ring); for the LAST chunk alternate with
                            # psbig — its ring is past the final exps by
                            # allocation order, so 4 units pipeline the tail.
                            pool, tg = ((psbig, "big")
                                        if cc == NCH - 1 and (tt + od) % 2
                                        else (psmid, "mid"))
                            yp = pool.tile([128, 512], F32, tag=tg,
                                           name=f"yp{cc}_{tt}_{od}")
                            for m in range(4):
                                nc.tensor.matmul(
                                    yp[:],
                                    ots[m][:, tt * 128:(tt + 1) * 128],
                                    wo_sb[:, m, od * 512:(od + 1) * 512],
                                    start=(m == 0), stop=(m == 3),
                                )
                            ysb = st2.tile([128, 512], F32, tag="y", bufs=6,
                                           name=f"ysb{cc}_{tt}_{od}")
                            nc.scalar.activation(out=ysb[:], in_=yp[:],
                                                 func=COPY)
                            t0 = cc * 512 + tt * 128
                            nc.sync.dma_start(
                                out=y_d.ap()[t0:t0 + 128,
                                             od * 512:(od + 1) * 512],
                                in_=ysb[:])
                        return emit
                    return [one(tt, od) for tt in range(4) for od in range(2)]

                outq.extend(make_outproj(c, ot_c))
            while outq:
                outq.pop(0)()

    nc.compile()
    return nc


_CACHE = {}


def _get_module(T, with_qkbias, with_vbias, n_cores):
    key = (T, with_qkbias, with_vbias, n_cores)
    if key not in _CACHE:
        _CACHE[key] = build_module(T, with_qkbias, with_vbias, n_cores)
    return _CACHE[key]


def make_consts(qn_w, kn_w):
    HG = 8
    tri = np.triu(np.ones((128, 128), np.float32))   # keep k<=q: [i <= j]
    # blk[p, j, r] = 1 where r == 32j + p//64: unit-j sum-of-squares
    # selector covering all 66 output rows (zeros elsewhere).
    blk = np.zeros((128, 3, 66), np.float32)
    for j in range(3):
        blk[0:64, j, 32 * j] = 1.0
        blk[64:128, j, 32 * j + 1] = 1.0
    # broadcast lhsT replicated at partition bases 0/32/64 (PE needs
    # lhsT and rhs at the same base); cols 0:128 = qn, 128:256 = kn.
    bcqk = np.zeros((66, 256), np.float32)
    for j in range(3):
        for half in range(2):
            bcqk[32 * j + half, half * 64:(half + 1) * 64] = qn_w
            bcqk[32 * j + half, 128 + half * 64:128 + (half + 1) * 64] = kn_w
    # 256: cancels the 8x*32W fp8 scaling of v through the softmax-denom
    # column (denom = 256*sum(es) meets numerator sum(es * 256*v)).
    vones = 256.0 * np.ones((128, HG), np.float32)
    return tri, blk, bcqk, vones


E4 = ml_dtypes.float8_e4m3


def _hilo8(a):
    hi = a.astype(E4)
    lo = (a - hi.astype(np.float32)).astype(E4)
    return hi.astype(np.float32), lo.astype(np.float32)


def pack_w8(wT):
    """wT [1024, 512] already scaled: DoubleRow main [4,2,128,512] (k-chunk
    pairs of hi) and correction [8,2,128,512] (per-chunk (hi, lo))."""
    hi, lo = _hilo8(wT)
    m = hi.reshape(4, 2, 128, 512)
    c = np.stack([hi.reshape(8, 128, 512), lo.reshape(8, 128, 512)], axis=1)
    return (np.ascontiguousarray(m).astype(E4),
            np.ascontiguousarray(c).astype(E4))


def pack_x8(xT, T):
    """xT [1024, T] scaled: main = hi pairs, corr packs (lo, hi)."""
    hi, lo = _hilo8(xT)
    m = hi.reshape(4, 2, 128, T)
    c = np.stack([lo.reshape(8, 128, T), hi.reshape(8, 128, T)], axis=1)
    return (np.ascontiguousarray(m).astype(E4),
            np.ascontiguousarray(c).astype(E4))


def make_in_maps(x, Wq, bq, Wk, bk, Wv, bv, Wo, qn_w, kn_w,
                 with_qkbias, with_vbias, n_cores=8):
    DG = 512
    tri, blk, bcqk, vones = make_consts(qn_w.astype(np.float32),
                                        kn_w.astype(np.float32))
    in_maps = []
    for c in range(n_cores):
        b, hg = divmod(c, 2)
        sl = slice(hg * DG, (hg + 1) * DG)
        bf = ml_dtypes.bfloat16
        T = x.shape[1]
        xm, xc = pack_x8(8.0 * x[b].T.astype(np.float32), T)
        wqm, wqc = pack_w8(32.0 * Wq[sl, :].T.astype(np.float32))
        wkm, wkc = pack_w8(32.0 * Wk[sl, :].T.astype(np.float32))
        wvm, wvc = pack_w8(32.0 * Wv[sl, :].T.astype(np.float32))
        im = {
            "xm": xm, "xc": xc,
            "wqm": wqm, "wqc": wqc,
            "wkm": wkm, "wkc": wkc,
            "wvm": wvm, "wvc": wvc,
            "wo": np.ascontiguousarray(Wo[:, sl].T.astype(bf)),
            "tri": tri.astype(ml_dtypes.bfloat16), "blk": blk, "bcqk": bcqk,
            "vones": vones.astype(ml_dtypes.bfloat16),
        }
        if with_qkbias:
            im["bq"] = 256.0 * bq[sl].astype(np.float32).reshape(4, 128)
            im["bk"] = 256.0 * bk[sl].astype(np.float32).reshape(4, 128)
        if with_vbias:
            im["bv"] = 256.0 * bv[sl].astype(np.float32).reshape(1, DG)
            im["ones1"] = np.ones((1, 128), np.float32)
        in_maps.append(im)
    return in_maps


_RUNNER_CACHE = {}


def _run_cached(nc, in_maps, key):
    """run_bass_via_pjrt with the jitted executable cached across calls."""
    import jax
    from jax.sharding import Mesh, PartitionSpec
    from jax.experimental.shard_map import shard_map
    from concourse import mybir as _mb

    n_cores = len(in_maps)
    if key not in _RUNNER_CACHE:
        bass2jax.install_neuronx_cc_hook()
        part_name = (nc.partition_id_tensor.name
                     if nc.partition_id_tensor else None)
        in_names, out_names, out_avals = [], [], []
        for alloc in nc.m.functions[0].allocations:
            if not isinstance(alloc, _mb.MemoryLocationSet):
                continue
            name = alloc.memorylocations[0].name
            if alloc.kind == "ExternalInput":
                if name != part_name:
                    in_names.append(name)
            elif alloc.kind == "ExternalOutput":
                out_names.append(name)
                out_avals.append(jax.core.ShapedArray(
                    tuple(alloc.tensor_shape), _mb.dt.np(alloc.dtype)))
        n_params = len(in_names)
        all_names = in_names + out_names
        if part_name is not None:
            all_names = all_names + [part_name]

        def _body(*args):
            operands = list(args)
            if part_name is not None:
                operands.append(bass2jax.partition_id_tensor())
            outs = bass2jax._bass_exec_p.bind(
                *operands, out_avals=tuple(out_avals),
                in_names=tuple(all_names), out_names=tuple(out_names),
                lowering_input_output_aliases=(),
                sim_require_finite=True, sim_require_nnan=True, nc=nc)
            return tuple(outs)

        devices = jax.devices()[:n_cores]
        mesh = Mesh(np.asarray(devices), ("core",))
        n_outs = len(out_names)
        sharded = jax.jit(
            shard_map(_body, mesh=mesh,
                      in_specs=(PartitionSpec("core"),) * (n_params + n_outs),
                      out_specs=(PartitionSpec("core"),) * n_outs,
                      check_rep=False),
            donate_argnums=tuple(range(n_params, n_params + n_outs)),
            keep_unused=True)
        _RUNNER_CACHE[key] = (sharded, in_names, out_names, out_avals)

    sharded, in_names, out_names, out_avals = _RUNNER_CACHE[key]
    concat_in = [np.concatenate([np.asarray(m[nm]) for m in in_maps], axis=0)
                 for nm in in_names]
    concat_zeros = [np.zeros((n_cores * a.shape[0], *a.shape[1:]), a.dtype)
                    for a in out_avals]
    out_arrs = sharded(*concat_in, *concat_zeros)
    return [
        {nm: np.asarray(out_arrs[i]).reshape(n_cores, *out_avals[i].shape)[c]
         for i, nm in enumerate(out_names)}
        for c in range(n_cores)
    ]


def kernel(x, Wq, bq, Wk, bk, Wv, bv, Wo, bo, qn_w, kn_w):
    x = np.asarray(x); Wq = np.asarray(Wq); Wk = np.asarray(Wk)
    Wv = np.asarray(Wv); Wo = np.asarray(Wo)
    bq = np.asarray(bq); bk = np.asarray(bk); bv = np.asarray(bv)
    bo = np.asarray(bo)
    qn_w = np.asarray(qn_w); kn_w = np.asarray(kn_w)
    B, T, D = x.shape

    with_qkbias = bool(np.any(bq != 0) or np.any(bk != 0))
    with_vbias = bool(np.any(bv != 0))
    nc = _get_module(T, with_qkbias, with_vbias, 8)
    in_maps = make_in_maps(x, Wq, bq, Wk, bk, Wv, bv, Wo, qn_w, kn_w,
                           with_qkbias, with_vbias, 8)
    key = (T, with_qkbias, with_vbias, 8)
    results = _run_cached(nc, in_maps, key)
    out = np.empty((B, T, D), np.float32)
    for b in range(B):
        out[b] = results[2 * b]["y"] + results[2 * b + 1]["y"]
    out += bo.astype(np.float32)
    return out



# revision 59
# speedup vs baseline: 1.0362x; 1.0005x over previous
"""Causal temporal attention (B=4, T=2048, D=1024, H=16, hd=64) on 8 trn2 cores.

Sharding: core c handles batch b=c//2 and head-group hg=c%2 (8 heads, 512 dims).
Each core computes y_partial[b] = attn_out_g @ Wo_g.T for its head group; the
host sums the two partials per batch and adds bo.

Per-core dataflow:
  xT [1024, 2048] (host-pretransposed x[b]) streams in 256-col sub-chunks.
  qT,kT are computed transposed [512, T] (dims on partitions) so the S matmul
  contracts head_dim on partitions; v is computed natural [T, 512] with an
  appended ones-column per head so the AV matmul also produces the softmax
  denominator (row 64 of the [65, 512] accumulator).
  RMS-norm over head_dim (= partitions) uses a block-ones matmul for the
  sum-of-squares, ln/exp on ACT for rsqrt, and a broadcast matmul (with the
  norm weight folded in) to spread it back over partitions.
  Causality: tiles above the diagonal are skipped; boundary 128x128 blocks
  are masked by a triangular 0/1 multiply on GPSIMD after the exp.
All matmul inputs are float32r (TF32-like rounding, fp32 accumulation).
"""

import ml_dtypes
import numpy as np

import concourse.bass as bass
import concourse.tile as tile
from concourse import bacc, mybir
from concourse.bass_utils import run_bass_kernel_spmd
from concourse import bass2jax

F32 = mybir.dt.float32
F32R = mybir.dt.float32r
BF16 = mybir.dt.bfloat16
FP8 = mybir.dt.float8e4
DR = mybir.MatmulPerfMode.DoubleRow
EXP = mybir.ActivationFunctionType.Exp
LN = mybir.ActivationFunctionType.Ln
COPY = mybir.ActivationFunctionType.Copy

EPS = 1e-6

# Force Ln and Exp onto the one ACT table set that contains both
# ("natural_log_exp_and_others"): the default first-match assignment puts
# them in different sets, and every Ln<->Exp transition then costs a ~2.7us
# table reload. Filtering (not reordering) keeps act_func_set_id positions
# valid for walrus.
_orig_gat = bacc.get_activation_tables


def _gat_combined(arch):
    tabs = _orig_gat(arch)
    drop = {mybir.ActivationFunctionType.Exp, mybir.ActivationFunctionType.Ln}
    return {
        name: (fns if name == "natural_log_exp_and_others" else fns - drop)
        for name, fns in tabs.items()
    }


bacc.get_activation_tables = _gat_combined


def build_module(T=2048, with_qkbias=False, with_vbias=False, n_cores=8):
    """Build the per-core Bass module. D=1024, 8 heads of 64 dims per core."""
    D = 1024
    HG = 8          # heads per core
    HD = 64         # head dim
    DG = HG * HD    # 512 group dims
    NKT = T // 128  # k/t tiles
    NCH = T // 512  # q chunks
    SUB = 256       # xT streaming sub-chunk width

    nc = bacc.Bacc("TRN2", target_bir_lowering=False, debug=False,
                   num_devices=n_cores)

    # fp8 DoubleRow projections: x scaled x8, W scaled x32 (values centered
    # in e4m3 range), decomposed host-side into hi + residual-lo. Main
    # matmuls contract k-chunk PAIRS of hi*hi; correction matmuls pack the
    # (hi*lo + lo*hi) cross terms. psum = 256*(x@W); q/k renormalize via
    # rmsnorm, v's 256 cancels against a 256-valued denominator column.
    xm_d = nc.dram_tensor("xm", [4, 2, 128, T], FP8, kind="ExternalInput")
    xc_d = nc.dram_tensor("xc", [8, 2, 128, T], FP8, kind="ExternalInput")
    wqm_d = nc.dram_tensor("wqm", [4, 2, 128, DG], FP8, kind="ExternalInput")
    wqc_d = nc.dram_tensor("wqc", [8, 2, 128, DG], FP8, kind="ExternalInput")
    wkm_d = nc.dram_tensor("wkm", [4, 2, 128, DG], FP8, kind="ExternalInput")
    wkc_d = nc.dram_tensor("wkc", [8, 2, 128, DG], FP8, kind="ExternalInput")
    wvm_d = nc.dram_tensor("wvm", [4, 2, 128, DG], FP8, kind="ExternalInput")
    wvc_d = nc.dram_tensor("wvc", [8, 2, 128, DG], FP8, kind="ExternalInput")
    wo_d = nc.dram_tensor("wo", [DG, D], BF16, kind="ExternalInput")
    tri_d = nc.dram_tensor("tri", [128, 128], BF16, kind="ExternalInput")
    blk_d = nc.dram_tensor("blk", [128, 3, 66], F32R, kind="ExternalInput")
    bcqk_d = nc.dram_tensor("bcqk", [66, 256], F32R, kind="ExternalInput")
    vones_d = nc.dram_tensor("vones", [128, HG], BF16, kind="ExternalInput")
    ones64_d = nc.dram_tensor("ones64", [1, 64], F32R, kind="ExternalInput")
    if with_qkbias:
        bq_d = nc.dram_tensor("bq", [4, 128], F32, kind="ExternalInput")
        bk_d = nc.dram_tensor("bk", [4, 128], F32, kind="ExternalInput")
    if with_vbias:
        bv_d = nc.dram_tensor("bv", [1, DG], F32R, kind="ExternalInput")
        ones1_d = nc.dram_tensor("ones1", [1, 128], F32R, kind="ExternalInput")
    y_d = nc.dram_tensor("y", [T, D], F32, kind="ExternalOutput")

    with nc.allow_low_precision(reason="float32r matmul inputs"), \
         tile.TileContext(nc) as tc:
        with (
            tc.tile_pool(name="res", bufs=1) as res,
            tc.tile_pool(name="ktp", bufs=1) as ktp,
            tc.tile_pool(name="vtp", bufs=1) as vtp,
            tc.tile_pool(name="st2", bufs=2) as st2,
            tc.tile_pool(name="st3", bufs=3) as st3,
            tc.tile_pool(name="st5", bufs=5) as st5,
            tc.tile_pool(name="qtp", bufs=2) as qtp,
            tc.tile_pool(name="psbig", bufs=2, space="PSUM") as psbig,
            tc.tile_pool(name="psmid", bufs=2, space="PSUM") as psmid,
            tc.tile_pool(name="pso", bufs=2, space="PSUM") as pso,
        ):
            # ---- resident loads ----
            # (xT chunk-0 and wq are hoisted first so the first projection
            # matmuls aren't stuck behind the full weight download)
            xm_ap = xm_d.ap().rearrange("a b p t -> p a b t")
            xc_ap = xc_d.ap().rearrange("a b p t -> p a b t")
            wqm_sb = res.tile([128, 4, 2, DG], FP8, tag="wqm")
            wqc_sb = res.tile([128, 8, 2, DG], FP8, tag="wqc")
            wkm_sb = res.tile([128, 4, 2, DG], FP8, tag="wkm")
            wkc_sb = res.tile([128, 8, 2, DG], FP8, tag="wkc")
            wvm_sb = res.tile([128, 4, 2, DG], FP8, tag="wvm")
            wvc_sb = res.tile([128, 8, 2, DG], FP8, tag="wvc")
            wo_sb = res.tile([128, 4, D], BF16, tag="wo")
            wqm_ap = wqm_d.ap().rearrange("a b p m -> p a b m")
            wqc_ap = wqc_d.ap().rearrange("a b p m -> p a b m")
            # fine-grained startup: per-k-pair pieces so the first projection
            # matmuls are gated on ~0.3MB of DMA instead of 3MB.
            xm0 = st3.tile([128, 4, 2, 512], FP8, tag="xm", bufs=3,
                           name="xm0")
            xc0 = st3.tile([128, 8, 2, 512], FP8, tag="xc", bufs=3,
                           name="xc0")
            for kp in range(4):
                nc.sync.dma_start(out=wqm_sb[:, kp], in_=wqm_ap[:, kp])
                nc.sync.dma_start(out=xm0[:, kp], in_=xm_ap[:, kp, :, 0:512])
            for kh in range(2):
                nc.sync.dma_start(out=wqc_sb[:, 4 * kh:4 * kh + 4],
                                  in_=wqc_ap[:, 4 * kh:4 * kh + 4])
                nc.sync.dma_start(out=xc0[:, 4 * kh:4 * kh + 4],
                                  in_=xc_ap[:, 4 * kh:4 * kh + 4, :, 0:512])
            xts0 = (xm0, xc0)
            nc.sync.dma_start(out=wkm_sb[:], in_=wkm_d.ap().rearrange("a b p m -> p a b m"))
            nc.sync.dma_start(out=wkc_sb[:], in_=wkc_d.ap().rearrange("a b p m -> p a b m"))
            nc.sync.dma_start(out=wvm_sb[:], in_=wvm_d.ap().rearrange("a b p m -> p a b m"))
            nc.sync.dma_start(out=wvc_sb[:], in_=wvc_d.ap().rearrange("a b p m -> p a b m"))
            tri_sb = res.tile([128, 128], BF16, tag="tri")
            nc.sync.dma_start(out=tri_sb[:], in_=tri_d.ap())
            blk_sb = res.tile([128, 3, 66], F32R, tag="blk")
            nc.sync.dma_start(out=blk_sb[:], in_=blk_d.ap())
            bcqk_sb = res.tile([66, 256], F32R, tag="bcqk")
            nc.sync.dma_start(out=bcqk_sb[:], in_=bcqk_d.ap())
            vones_sb = res.tile([128, HG], BF16, tag="vones")
            nc.sync.dma_start(out=vones_sb[:], in_=vones_d.ap())
            eps_sb = res.tile([66, 1], F32, tag="eps")
            nc.vector.memset(eps_sb[:], EPS)
            ones64 = res.tile([1, 64], F32R, tag="ones64")
            nc.sync.dma_start(out=ones64[:], in_=ones64_d.ap())
            nc.sync.dma_start(out=wo_sb[:], in_=wo_d.ap().rearrange("(a p) m -> p a m", p=128))
            bq_sb = bk_sb = bv_sb = ones1_sb = None
            if with_qkbias:
                bq_sb = res.tile([128, 4], F32, tag="bq")
                nc.sync.dma_start(out=bq_sb[:], in_=bq_d.ap().rearrange("m p -> p m"))
                bk_sb = res.tile([128, 4], F32, tag="bk")
                nc.sync.dma_start(out=bk_sb[:], in_=bk_d.ap().rearrange("m p -> p m"))
            if with_vbias:
                bv_sb = res.tile([1, DG], F32R, tag="bv")
                nc.sync.dma_start(out=bv_sb[:], in_=bv_d.ap())
                ones1_sb = res.tile([1, 128], F32R, tag="ones1")
                nc.sync.dma_start(out=ones1_sb[:], in_=ones1_d.ap())

            # resident kT [dims, T] (4 tiles) and v [t, dims+ones] (NKT tiles)
            kt_sb = [ktp.tile([128, T], BF16, tag=f"kt{m}", name=f"kt{m}") for m in range(4)]
            v_sb = [vtp.tile([128, HG, HD + 1], BF16, tag=f"v{t}", name=f"v{t}")
                    for t in range(NKT)]

            # filler queues: ~0.4-0.9us closures of pure PE work, popped
            # wherever the PE stream would otherwise stall (proj-phase DMA
            # waits, attention exp waits). vfills entries are (chunk, fn);
            # chunk cc's vfills must all be emitted by the end of attention
            # cc-1 (the diagonal AV of chunk cc reads its own v tiles).
            vfills = []
            outq = []
            xts_map = {0: xts0}

            def pop_fill(keep=0):
                if vfills:
                    vfills.pop(0)[1]()
                elif len(outq) > keep:
                    outq.pop(0)()

            def make_vfills(cc, xts_use):
                """v = x@Wv in fp8 DoubleRow, split in two ~0.9us halves:
                half 0 = main hi*hi over k-pairs, half 1 = corrections."""
                xm_t, xc_t = xts_use
                vstate = {}

                def half(tt, lo):
                    def emit():
                        toff = tt * 128
                        if lo == 0:
                            vstate[tt] = psmid.tile(
                                [128, 512], F32, tag="mid",
                                name=f"vps{cc}_{tt}")
                            ps = vstate[tt]
                            for kp in range(4):
                                nc.tensor.matmul(
                                    ps[:], xm_t[:, kp, :, toff:toff + 128],
                                    wvm_sb[:, kp], perf_mode=DR,
                                    start=(kp == 0), stop=False)
                        else:
                            ps = vstate[tt]
                            for k in range(8):
                                nc.tensor.matmul(
                                    ps[:], xc_t[:, k, :, toff:toff + 128],
                                    wvc_sb[:, k], perf_mode=DR,
                                    start=False,
                                    stop=(k == 7 and not with_vbias))
                            if with_vbias:
                                nc.tensor.matmul(
                                    ps[:], ones1_sb[:], bv_sb[:],
                                    start=False, stop=True)
                            vt = v_sb[cc * 4 + tt]
                            nc.vector.tensor_copy(
                                vt[:, :, 0:HD],
                                ps[:].rearrange("p (h d) -> p h d", h=HG),
                            )
                            nc.sync.dma_start(
                                out=vt[:, :, HD:HD + 1],
                                in_=vones_sb[:].rearrange(
                                    "p (h o) -> p h o", o=1),
                            )
                    return emit

                return [half(tt, lo) for tt in range(4) for lo in (0, 4)]

            def stage(cc):
                """Prefetch chunk cc's x (fp8 main+corr) and queue its
                v-proj fillers."""
                if cc >= NCH or cc in xts_map:
                    return
                c0 = cc * 512
                xm_t = st3.tile([128, 4, 2, 512], FP8, tag="xm", bufs=3,
                                name=f"xm{cc}")
                nc.sync.dma_start(out=xm_t[:], in_=xm_ap[:, :, :, c0:c0 + 512])
                xc_t = st3.tile([128, 8, 2, 512], FP8, tag="xc", bufs=3,
                                name=f"xc{cc}")
                nc.sync.dma_start(out=xc_t[:], in_=xc_ap[:, :, :, c0:c0 + 512])
                xts_map[cc] = (xm_t, xc_t)
                vfills.extend((cc, fn) for fn in make_vfills(cc, (xm_t, xc_t)))

            for c in range(NCH):
                # ---------- projection phase for chunk c ----------
                xts = xts_map[c]

                qt_c = [qtp.tile([128, 512], BF16, tag=f"qt{m}", name=f"qt{m}", bufs=1)
                        for m in range(4)]

                # projection pipeline stages, skewed so PE never waits on
                # the DVE/ACT legs of the rms-norm chain.
                praw, psq = {}, {}

                def proj_qk(u):
                    which, m = u
                    wm = wqm_sb if which == "q" else wkm_sb
                    wc = wqc_sb if which == "q" else wkc_sb
                    b_sb = bq_sb if which == "q" else bk_sb
                    xm_t, xc_t = xts
                    mc = slice(m * 128, (m + 1) * 128)
                    ps = psmid.tile([128, 512], F32, tag="mid")
                    for kp in range(4):
                        nc.tensor.matmul(ps[:], wm[:, kp, :, mc],
                                         xm_t[:, kp], perf_mode=DR,
                                         start=(kp == 0), stop=False)
                    for k in range(8):
                        nc.tensor.matmul(ps[:], wc[:, k, :, mc],
                                         xc_t[:, k], perf_mode=DR,
                                         start=False, stop=(k == 7))
                    raw = st5.tile([128, 512], F32, tag="praw", bufs=8)
                    if b_sb is not None:
                        nc.vector.tensor_scalar_add(raw[:], ps[:],
                                                    b_sb[:, m:m + 1])
                    else:
                        # psum->sbuf copies ride on ACT: DVE is the backlog
                        # engine at phase boundaries (recip chains + v/qt
                        # muls), and ACT has slack outside the exp bursts.
                        nc.scalar.activation(out=raw[:], in_=ps[:], func=COPY)
                    sq = st3.tile([128, 512], F32R, tag="sq", bufs=1)
                    nc.vector.tensor_mul(sq[:], raw[:], raw[:])
                    praw[u] = raw
                    psq[u] = sq

                # rsqrt staging: units packed 3-per-tile at 32-aligned
                # partition bases (matmul bases must be 0/32/64). The ln/exp
                # run over the whole [66, 512] tile; rows between the packed
                # pairs are junk that is never read.
                rs_tiles = [st2.tile([66, 512], F32R, tag=f"rs{j}",
                                     name=f"rs{j}", bufs=1) for j in range(3)]
                ssq3 = [None, None, None]

                def rs_slice(i):
                    return rs_tiles[i // 3][32 * (i % 3):32 * (i % 3) + 2, :]

                def sumsq(i, u):
                    g, j = divmod(i, 3)
                    if j == 0:
                        ssq3[g] = pso.tile([66, 512], F32, tag="o",
                                           name=f"ssq3_{g}")
                    last = i in (2, 5, 7)
                    # blk3[:, j] spreads unit j's sums to rows 32j:32j+2 and
                    # zeros elsewhere, so the accumulated tile is fully
                    # written before the ln reads it.
                    nc.tensor.matmul(ssq3[g][:], blk_sb[:, j, :],
                                     psq[u][:], start=(j == 0), stop=last)
                    if last:
                        # ln then rsqrt-exp immediately: the rs chain for
                        # group g completes while later units still project,
                        # so the first bcast_mul never waits on ACT.
                        nc.scalar.activation(out=rs_tiles[g][:],
                                             in_=ssq3[g][:], func=LN,
                                             bias=eps_sb[:], scale=1.0 / HD)
                        nc.scalar.activation(out=rs_tiles[g][:],
                                             in_=rs_tiles[g][:],
                                             func=EXP, scale=-0.5)

                def bcast_mul(i, u):
                    which, m = u
                    rsb = psbig.tile([128, 512], F32, tag="big")
                    b0 = 32 * (i % 3)
                    co = 0 if which == "q" else 128
                    nc.tensor.matmul(rsb[:],
                                     bcqk_sb[b0:b0 + 2, co:co + 128],
                                     rs_slice(i),
                                     start=True, stop=True)
                    if which == "q":
                        nc.vector.tensor_mul(qt_c[m][:], praw[u][:], rsb[:])
                    else:
                        nc.vector.tensor_mul(
                            kt_sb[m][:, c * 512:(c + 1) * 512],
                            praw[u][:], rsb[:])

                def proj_v(tt, cc, xts_use):
                    xm_t, xc_t = xts_use
                    toff = tt * 128
                    ps = psmid.tile([128, 512], F32, tag="mid")
                    for kp in range(4):
                        nc.tensor.matmul(
                            ps[:], xm_t[:, kp, :, toff:toff + 128],
                            wvm_sb[:, kp], perf_mode=DR,
                            start=(kp == 0), stop=False)
                    for k in range(8):
                        nc.tensor.matmul(
                            ps[:], xc_t[:, k, :, toff:toff + 128],
                            wvc_sb[:, k], perf_mode=DR,
                            start=False, stop=(k == 7 and not with_vbias))
                    if with_vbias:
                        nc.tensor.matmul(ps[:], ones1_sb[:], bv_sb[:],
                                         start=False, stop=True)
                    vt = v_sb[cc * 4 + tt]
                    nc.vector.tensor_copy(
                        vt[:, :, 0:HD],
                        ps[:].rearrange("p (h d) -> p h d", h=HG),
                    )
                    nc.sync.dma_start(
                        out=vt[:, :, HD:HD + 1],
                        in_=vones_sb[:].rearrange("p (h o) -> p h o", o=1),
                    )

                units = [("q", m) for m in range(4)] + [("k", m) for m in range(4)]
                # proj(u_i) skewed with sumsq(u_{i-1}); the previous chunk's
                # deferred out-projection interleaves here (queues are quiet);
                # then v tiles (PE work covering the ACT ln/exp latency);
                # then the 8 bcast+muls.
                # no pops in the unit loop: keep the mid psum ring free for
                # the projection pipeline. Fillers drain in the attention
                # phase, where the exp stream leaves PE slack.
                for i, u in enumerate(units):
                    proj_qk(u)
                    if i >= 1:
                        sumsq(i - 1, units[i - 1])
                sumsq(len(units) - 1, units[-1])
                # all 8 norm-broadcasts here in the proj phase: the big ring
                # is idle (attention hasn't started), so rsb never steals an
                # S-pipeline slot mid-attention.
                for mt in range(4):
                    bcast_mul(mt, ("q", mt))
                    bcast_mul(4 + mt, ("k", mt))
                if c == 0:
                    for tt in range(4):
                        proj_v(tt, 0, xts)

                # ---------- attention phase for chunk c ----------
                # prefetch TWO chunks ahead: chunk c+2's v-proj fillers give
                # the late stretch of this attention phase (after c+1's
                # fillers run out) more PE supply.
                stage(c + 1)
                stage(c + 2)
                # bufs=2: deferred out-projection closures may emit after the
                # NEXT chunk's attention starts writing its ot tiles; a ring
                # of 2 keeps the reads on the old slot.
                ot_c = [qtp.tile([128, 512], BF16, tag=f"ot{m}", name=f"ot{m}", bufs=2)
                        for m in range(4)]
                fill_tick = 0
                # last chunk: keep 2 out-proj fillers in the queue for the
                # final ot3 wait (they emit first in the tail drain).
                keep_c = 2 if c == NCH - 1 else 0

                def tick():
                    nonlocal fill_tick
                    fill_tick += 1
                    if fill_tick % 4 == 0:
                        pop_fill(keep_c)

                # Flattened attention pipeline: all heads' S/exp/AV groups in
                # one stream, with the AV lag carried ACROSS head boundaries
                # so the first AV of a head never waits on its own first exp.
                # Groups: per head, pairs of full k-tiles, then 2 diagonal
                # groups packing the 4 boundary tiles (masked post-exp).
                n_full = 4 * c
                chunk_groups = []
                for h in range(HG):
                    mt, r0 = h // 2, (h % 2) * 64
                    glist = []
                    for p0 in range(0, n_full, 2):
                        sm = [(0, p0, 0, 512), (512, p0 + 1, 0, 512)]
                        glist.append(dict(
                            smm=sm, etot=1024, mask=None,
                            av=[sm[0] + (p0 == 0, False),
                                sm[1] + (False, False)]))
                    ga = (0, n_full + 0, 0, 512)
                    gb = (512, n_full + 2, 256, 256)
                    glist.append(dict(
                        smm=[ga, gb], etot=768, mask=4,
                        av=[ga + (n_full == 0, False), gb + (False, False)]))
                    gc_ = (0, n_full + 1, 128, 384)
                    gd = (384, n_full + 3, 384, 128)
                    glist.append(dict(
                        smm=[gc_, gd], etot=512, mask=3,
                        av=[gc_ + (False, False), gd + (False, True)]))
                    for gi, g in enumerate(glist):
                        g.update(h=h, mt=mt, r0=r0,
                                 first_of_head=(gi == 0),
                                 last_of_head=(gi == len(glist) - 1))
                        chunk_groups.append(g)

                o_ps_map = {}

                def finish_head(g):
                    h, mt, r0 = g["h"], g["mt"], g["r0"]
                    o_ps = o_ps_map[h]
                    recip = st2.tile([1, 512], F32R, tag="recip", bufs=1)
                    nc.vector.reciprocal(out=recip[:], in_=o_ps[64:65, :])
                    if c == NCH - 1 and h == HG - 1:
                        # final head: the whole tail waits on this chain, so
                        # broadcast on PE (idle here) instead of GPSIMD. DVE
                        # may read only ONE psum operand, so stage o_ps rows
                        # to SBUF on ACT (concurrent with the recip).
                        rb_ps = pso.tile([64, 512], F32, tag="o",
                                         name="rb_ps")
                        nc.tensor.matmul(rb_ps[:], ones64[:], recip[:],
                                         start=True, stop=True)
                        osb = st2.tile([64, 512], F32R, tag="recipb",
                                       bufs=1, name="osb_last")
                        nc.scalar.activation(out=osb[:], in_=o_ps[0:64, :],
                                             func=COPY)
                        nc.vector.tensor_mul(ot_c[mt][r0:r0 + 64, :],
                                             osb[:], rb_ps[:])
                    else:
                        recipb = st2.tile([64, 512], F32R, tag="recipb",
                                          bufs=1)
                        nc.gpsimd.partition_broadcast(recipb[:], recip[:])
                        nc.vector.tensor_mul(ot_c[mt][r0:r0 + 64, :],
                                             o_ps[0:64, :], recipb[:])

                def flush(p):
                    g, es = p
                    o_ps = o_ps_map[g["h"]]
                    for (col0, kt, q0, w, fi, la) in g["av"]:
                        nc.tensor.matmul(
                            o_ps[:, q0:q0 + w],
                            v_sb[kt][:, g["h"], :],
                            es[:, col0:col0 + w],
                            start=fi, stop=la,
                        )
                    if g["last_of_head"]:
                        finish_head(g)

                pend = None
                for g in chunk_groups:
                    if g["first_of_head"]:
                        pop_fill(keep_c)
                        o_ps_map[g["h"]] = pso.tile(
                            [HD + 1, 512], F32, tag="o",
                            name=f"o_ps{c}_{g['h']}")
                    sp = psbig.tile([128, 1024], F32, tag="big")
                    for (col0, kt, q0, width) in g["smm"]:
                        nc.tensor.matmul(
                            sp[:, col0:col0 + width],
                            kt_sb[g["mt"]][g["r0"]:g["r0"] + 64,
                                           kt * 128:(kt + 1) * 128],
                            qt_c[g["mt"]][g["r0"]:g["r0"] + 64,
                                          q0:q0 + width],
                            start=True, stop=True,
                        )
                    es = st3.tile([128, 1024], BF16, tag="es", bufs=3)
                    nc.scalar.activation(out=es[:, 0:g["etot"]],
                                         in_=sp[:, 0:g["etot"]],
                                         func=EXP, scale=0.125)
                    if g["mask"] is not None:
                        bstep = g["mask"]
                        esb = es[:].rearrange("p (a w) -> p a w", w=128)
                        nc.vector.tensor_mul(
                            esb[:, 0:bstep + 1:bstep, :],
                            esb[:, 0:bstep + 1:bstep, :],
                            tri_sb[:].rearrange("p (o w) -> p o w", o=1)
                            .to_broadcast((128, 2, 128)),
                        )
                    if pend is not None:
                        flush(pend)
                        tick()
                    pend = (g, es)
                flush(pend)
                # next chunk's attention needs its v tiles from the first
                # head's diagonal groups on: drain chunk c+1's leftovers
                # (chunk c+2's may linger into the next attention phase).
                while vfills and vfills[0][0] <= c + 1:
                    vfills.pop(0)[1]()

                # ---------- out-projection for chunk c (deferred) ----------
                def make_outproj(cc, ots):
                    def one(tt, od):
                        def emit():
                            # psmid while attention phases follow (psbig is
                            # the S-ring); for the LAST chunk alternate with
                            # psbig — its ring is past the final exps by
                            # allocation order, so 4 units pipeline the tail.
                            pool, tg = ((psbig, "big")
                                        if cc == NCH - 1 and (tt + od) % 2
                                        else (psmid, "mid"))
                            yp = pool.tile([128, 512], F32, tag=tg,
                                           name=f"yp{cc}_{tt}_{od}")
                            for m in range(4):
                                nc.tensor.matmul(
                                    yp[:],
                                    ots[m][:, tt * 128:(tt + 1) * 128],
                                    wo_sb[:, m, od * 512:(od + 1) * 512],
                                    start=(m == 0), stop=(m == 3),
                                )
                            ysb = st2.tile([128, 512], F32, tag="y", bufs=6,
                                           name=f"ysb{cc}_{tt}_{od}")
                            nc.scalar.activation(out=ysb[:], in_=yp[:],
                                                 func=COPY)
                            t0 = cc * 512 + tt * 128
                            nc.sync.dma_start(
                                out=y_d.ap()[t0:t0 + 128,
                                             od * 512:(od + 1) * 512],
                                in_=ysb[:])
                        return emit
                    return [one(tt, od) for tt in range(4) for od in range(2)]

                outq.extend(make_outproj(c, ot_c))
            while outq:
                outq.pop(0)()

    nc.compile()
    return nc


_CACHE = {}


def _get_module(T, with_qkbias, with_vbias, n_cores):
    key = (T, with_qkbias, with_vbias, n_cores)
    if key not in _CACHE:
        _CACHE[key] = build_module(T, with_qkbias, with_vbias, n_cores)
    return _CACHE[key]


def make_consts(qn_w, kn_w):
    HG = 8
    tri = np.triu(np.ones((128, 128), np.float32))   # keep k<=q: [i <= j]
    # blk[p, j, r] = 1 where r == 32j + p//64: unit-j sum-of-squares
    # selector covering all 66 output rows (zeros elsewhere).
    blk = np.zeros((128, 3, 66), np.float32)
    for j in range(3):
        blk[0:64, j, 32 * j] = 1.0
        blk[64:128, j, 32 * j + 1] = 1.0
    # broadcast lhsT replicated at partition bases 0/32/64 (PE needs
    # lhsT and rhs at the same base); cols 0:128 = qn, 128:256 = kn.
    bcqk = np.zeros((66, 256), np.float32)
    for j in range(3):
        for half in range(2):
            bcqk[32 * j + half, half * 64:(half + 1) * 64] = qn_w
            bcqk[32 * j + half, 128 + half * 64:128 + (half + 1) * 64] = kn_w
    # 256: cancels the 8x*32W fp8 scaling of v through the softmax-denom
    # column (denom = 256*sum(es) meets numerator sum(es * 256*v)).
    vones = 256.0 * np.ones((128, HG), np.float32)
    return tri, blk, bcqk, vones


E4 = ml_dtypes.float8_e4m3


def _hilo8(a):
    hi = a.astype(E4)
    lo = (a - hi.astype(np.float32)).astype(E4)
    return hi.astype(np.float32), lo.astype(np.float32)


def pack_w8(wT):
    """wT [1024, 512] already scaled: DoubleRow main [4,2,128,512] (k-chunk
    pairs of hi) and correction [8,2,128,512] (per-chunk (hi, lo))."""
    hi, lo = _hilo8(wT)
    m = hi.reshape(4, 2, 128, 512)
    c = np.stack([hi.reshape(8, 128, 512), lo.reshape(8, 128, 512)], axis=1)
    return (np.ascontiguousarray(m).astype(E4),
            np.ascontiguousarray(c).astype(E4))


def pack_x8(xT, T):
    """xT [1024, T] scaled: main = hi pairs, corr packs (lo, hi)."""
    hi, lo = _hilo8(xT)
    m = hi.reshape(4, 2, 128, T)
    c = np.stack([lo.reshape(8, 128, T), hi.reshape(8, 128, T)], axis=1)
    return (np.ascontiguousarray(m).astype(E4),
            np.ascontiguousarray(c).astype(E4))


def make_in_maps(x, Wq, bq, Wk, bk, Wv, bv, Wo, qn_w, kn_w,
                 with_qkbias, with_vbias, n_cores=8):
    DG = 512
    tri, blk, bcqk, vones = make_consts(qn_w.astype(np.float32),
                                        kn_w.astype(np.float32))
    in_maps = []
    for c in range(n_cores):
        b, hg = divmod(c, 2)
        sl = slice(hg * DG, (hg + 1) * DG)
        bf = ml_dtypes.bfloat16
        T = x.shape[1]
        xm, xc = pack_x8(8.0 * x[b].T.astype(np.float32), T)
        wqm, wqc = pack_w8(32.0 * Wq[sl, :].T.astype(np.float32))
        wkm, wkc = pack_w8(32.0 * Wk[sl, :].T.astype(np.float32))
        wvm, wvc = pack_w8(32.0 * Wv[sl, :].T.astype(np.float32))
        im = {
            "xm": xm, "xc": xc,
            "wqm": wqm, "wqc": wqc,
            "wkm": wkm, "wkc": wkc,
            "wvm": wvm, "wvc": wvc,
            "wo": np.ascontiguousarray(Wo[:, sl].T.astype(bf)),
            "tri": tri.astype(ml_dtypes.bfloat16), "blk": blk, "bcqk": bcqk,
            "vones": vones.astype(ml_dtypes.bfloat16),
            "ones64": np.ones((1, 64), np.float32),
        }
        if with_qkbias:
            im["bq"] = 256.0 * bq[sl].astype(np.float32).reshape(4, 128)
            im["bk"] = 256.0 * bk[sl].astype(np.float32).reshape(4, 128)
        if with_vbias:
            im["bv"] = 256.0 * bv[sl].astype(np.float32).reshape(1, DG)
            im["ones1"] = np.ones((1, 128), np.float32)
        in_maps.append(im)
    return in_maps


_RUNNER_CACHE = {}


def _run_cached(nc, in_maps, key):
    """run_bass_via_pjrt with the jitted executable cached across calls."""
    import jax
    from jax.sharding import Mesh, PartitionSpec
    from jax.experimental.shard_map import shard_map
    from concourse import mybir as _mb

    n_cores = len(in_maps)
    if key not in _RUNNER_CACHE:
        bass2jax.install_neuronx_cc_hook()
        part_name = (nc.partition_id_tensor.name
                     if nc.partition_id_tensor else None)
        in_names, out_names, out_avals = [], [], []
        for alloc in nc.m.functions[0].allocations:
            if not isinstance(alloc, _mb.MemoryLocationSet):
                continue
            name = alloc.memorylocations[0].name
            if alloc.kind == "ExternalInput":
                if name != part_name:
                    in_names.append(name)
            elif alloc.kind == "ExternalOutput":
                out_names.append(name)
                out_avals.append(jax.core.ShapedArray(
                    tuple(alloc.tensor_shape), _mb.dt.np(alloc.dtype)))
        n_params = len(in_names)
        all_names = in_names + out_names
        if part_name is not None:
            all_names = all_names + [part_name]

        def _body(*args):
            operands = list(args)
            if part_name is not None:
                operands.append(bass2jax.partition_id_tensor())
            outs = bass2jax._bass_exec_p.bind(
                *operands, out_avals=tuple(out_avals),
                in_names=tuple(all_names), out_names=tuple(out_names),
                lowering_input_output_aliases=(),
                sim_require_finite=True, sim_require_nnan=True, nc=nc)
            return tuple(outs)

        devices = jax.devices()[:n_cores]
        mesh = Mesh(np.asarray(devices), ("core",))
        n_outs = len(out_names)
        sharded = jax.jit(
            shard_map(_body, mesh=mesh,
                      in_specs=(PartitionSpec("core"),) * (n_params + n_outs),
                      out_specs=(PartitionSpec("core"),) * n_outs,
                      check_rep=False),
            donate_argnums=tuple(range(n_params, n_params + n_outs)),
            keep_unused=True)
        _RUNNER_CACHE[key] = (sharded, in_names, out_names, out_avals)

    sharded, in_names, out_names, out_avals = _RUNNER_CACHE[key]
    concat_in = [np.concatenate([np.asarray(m[nm]) for m in in_maps], axis=0)
                 for nm in in_names]
    concat_zeros = [np.zeros((n_cores * a.shape[0], *a.shape[1:]), a.dtype)
                    for a in out_avals]
    out_arrs = sharded(*concat_in, *concat_zeros)
    return [
        {nm: np.asarray(out_arrs[i]).reshape(n_cores, *out_avals[i].shape)[c]
         for i, nm in enumerate(out_names)}
        for c in range(n_cores)
    ]


def kernel(x, Wq, bq, Wk, bk, Wv, bv, Wo, bo, qn_w, kn_w):
    x = np.asarray(x); Wq = np.asarray(Wq); Wk = np.asarray(Wk)
    Wv = np.asarray(Wv); Wo = np.asarray(Wo)
    bq = np.asarray(bq); bk = np.asarray(bk); bv = np.asarray(bv)
    bo = np.asarray(bo)
    qn_w = np.asarray(qn_w); kn_w = np.asarray(kn_w)
    B, T, D = x.shape

    with_qkbias = bool(np.any(bq != 0) or np.any(bk != 0))
    with_vbias = bool(np.any(bv != 0))
    nc = _get_module(T, with_qkbias, with_vbias, 8)
    in_maps = make_in_maps(x, Wq, bq, Wk, bk, Wv, bv, Wo, qn_w, kn_w,
                           with_qkbias, with_vbias, 8)
    key = (T, with_qkbias, with_vbias, 8)
    results = _run_cached(nc, in_maps, key)
    out = np.empty((B, T, D), np.float32)
    for b in range(B):
        out[b] = results[2 * b]["y"] + results[2 * b + 1]["y"]
    out += bo.astype(np.float32)
    return out

